# revision 23
# baseline (speedup 1.0000x reference)
"""GNN message-passing kernel for Trainium2 (8 NeuronCores, Bass/Tile).

Implements:
  embeds = node_emb[nodes] + features @ feat_W.T + feat_b
  2 x { agg[dst] += embeds[src];  embeds = relu((embeds+agg) @ conv1_W.T + conv1_b) }
  out = cosine(embeds, pattern_emb[pattern_id])

Distribution: nodes are permuted (in-degree balanced) and sharded over 8 cores
(12544 nodes each).  Edges are partitioned by dst owner; messages are fetched
with GPSIMD dma_gather (4 SWDGE queues, one per table chunk) from a replicated
bf16 hi|lo DRAM table rebuilt each round with an ncfw AllGather.  Per 128-dst
block the segment-sum is a one-hot matmul accumulated in PSUM.
"""

import os
import sys

sys.path.insert(0, "/opt/trn_rl_repo")

import numpy as np

import concourse.bass as bass
import concourse.bacc as bacc
import concourse.mybir as mybir
import concourse.tile as tile
from concourse import library_config
from concourse.bass_utils import run_bass_kernel_spmd

N = 100000
NP = 100352          # padded: 8 * 12544
W = 8                # cores
SH = NP // W         # 12544 nodes per core
NBLK = SH // 128     # 98 blocks per core
NBG = NP // 128      # 784 global blocks
NCHUNK = 4
CH = NP // NCHUNK    # 25088 rows per gather chunk (int16-safe)
D = 64
DF = 10
VOCAB = 100
EPS = 1e-8
CALL = int(os.environ.get("BASS_GNN_CALL", "4096"))          # gather positions per dma_gather call
G_OH = 16            # one-hot tiles built per DVE op
PHASE = os.environ.get("BASS_GNN_PHASE", "full")
TRACE = os.environ.get("BASS_GNN_TRACE", "0") == "1"

f32 = mybir.dt.float32
bf16 = mybir.dt.bfloat16
i16 = mybir.dt.int16
Alu = mybir.AluOpType
Act = mybir.ActivationFunctionType


# ----------------------------------------------------------------------------
# Host-side preprocessing: permutation, block assignment, edge streams.
# ----------------------------------------------------------------------------

def _prep(nodes, edges):
    src = edges[:, 0].astype(np.int64)
    dst = edges[:, 1].astype(np.int64)

    indeg = np.zeros(NP, np.int64)
    np.add.at(indeg, dst, 1)
    order = np.argsort(-indeg, kind="stable")
    # round-robin by in-degree over 784 global blocks -> balanced block loads
    gblock = np.empty(NP, np.int64)
    gslot = np.empty(NP, np.int64)
    gblock[order] = np.arange(NP) % NBG
    gslot[order] = np.arange(NP) // NBG

    # block -> core is FIXED (g % W) so each node's gather chunk (= its core
    # pair) is known exactly; then align per-(block,chunk) tile needs across
    # cores by sorting each core's blocks by their tile-need pattern.
    core_of_gblock = np.arange(NBG) % W
    chunk_n = core_of_gblock[gblock] // 2       # chunk of node as gather src
    E0 = np.zeros((NBG, NCHUNK), np.int64)
    np.add.at(E0, (gblock[dst], chunk_n[src]), 1)
    t0 = np.ceil(E0 / 128).astype(np.int64)
    key = (t0 * (5 ** np.arange(NCHUNK))[::-1]).sum(1) * 1000 + E0.sum(1) // 16
    lslot_of_gblock = np.empty(NBG, np.int64)
    for c in range(W):
        mine = np.where(core_of_gblock == c)[0]
        order_c = mine[np.argsort(key[mine], kind="stable")]
        lslot_of_gblock[order_c] = np.arange(len(order_c))

    # final permutation: pi(n) = core*SH + slot_within_block*NBLK + local_block
    # (p-major layout so SBUF slab [128, 98, 64] maps contiguously to DRAM rows)
    c_n = core_of_gblock[gblock]
    b_n = lslot_of_gblock[gblock]
    p_n = gslot
    pi = c_n * SH + p_n * NBLK + b_n

    # exact per-(core, local block, chunk) edge counts under final assignment
    e_dc = c_n[dst]
    e_db = b_n[dst]
    e_dp = p_n[dst]
    e_chunk = pi[src] // CH
    Ec = np.zeros((W, NBLK, NCHUNK), np.int64)
    np.add.at(Ec, (e_dc, e_db, e_chunk), 1)
    S_sub = np.ceil(Ec / 128).astype(np.int64).max(0)  # [NBLK, NCHUNK]
    S_sub = np.maximum(S_sub, 1)   # keep schedule uniform & non-degenerate
    assert S_sub.max() <= CALL // 128

    # edge order: (core, block, chunk, src)
    ek = np.lexsort((pi[src], e_chunk, e_db, e_dc))
    src_pi = pi[src][ek]
    dloc_e = e_dp[ek]
    key_cbk = (e_dc[ek] * NBLK + e_db[ek]) * NCHUNK + e_chunk[ek]
    counts = np.zeros(W * NBLK * NCHUNK + 1, np.int64)
    np.add.at(counts, key_cbk + 1, 1)
    starts = np.cumsum(counts)

    # gather-stream offsets: chunk-major, then block, then tile
    TPOS = int(S_sub.sum() * 128)
    NT = TPOS // 128
    sched_off = np.zeros((NCHUNK, NBLK), np.int64)
    off = 0
    for k in range(NCHUNK):
        for b in range(NBLK):
            sched_off[k, b] = off
            off += int(S_sub[b, k]) * 128
    assert off == TPOS

    gidx = np.zeros((W, TPOS), np.int32)
    dloc = np.full((W, TPOS), -1.0, np.float32)   # stream order
    for c in range(W):
        for k in range(NCHUNK):
            for b in range(NBLK):
                i0 = starts[(c * NBLK + b) * NCHUNK + k]
                i1 = starts[(c * NBLK + b) * NCHUNK + k + 1]
                n = i1 - i0
                o = sched_off[k, b]
                assert n <= S_sub[b, k] * 128
                gidx[c, o:o + n] = src_pi[i0:i1] - k * CH
                dloc[c, o:o + n] = dloc_e[i0:i1]
    assert gidx.min() >= 0 and gidx.max() < CH

    # wrapped-16 int16 idx layout (stream order)
    gidx16 = np.zeros((W, 128, TPOS // 16), np.int16)
    for c in range(W):
        g = gidx[c].reshape(-1, 16).T.astype(np.int16)
        gidx16[c] = np.tile(g, (8, 1))

    # consumption order: block-major (b, k, s) -> stream position of each tile
    cons_pos = []
    for b in range(NBLK):
        for k in range(NCHUNK):
            for s in range(int(S_sub[b, k])):
                cons_pos.append(int(sched_off[k, b]) + s * 128)
    cons_pos = np.array(cons_pos, np.int64)
    assert len(cons_pos) == NT
    # dloc in consumption-tile-column layout [128, NT]
    dloc_s = dloc.reshape(W, NT, 128)        # stream tile t at rows [t]
    stream_tile = cons_pos // 128
    dloc_c = dloc_s[:, stream_tile, :].transpose(0, 2, 1).astype(np.float32).copy()

    # per-chunk call plan, issued round-robin across chunks
    ncalls_k = []
    base_k = []
    for k in range(NCHUNK):
        npos = int(S_sub[:, k].sum() * 128)
        base_k.append(int(sched_off[k, 0]))
        ncalls_k.append((npos + CALL - 1) // CALL)
    calls = []   # (k, stream_off, npos) in issue order
    for ci in range(max(ncalls_k)):
        for k in range(NCHUNK):
            if ci < ncalls_k[k]:
                npos_k = int(S_sub[:, k].sum() * 128)
                o = base_k[k] + ci * CALL
                n = min(CALL, base_k[k] + npos_k - o)
                calls.append((k, o, n))

    return {
        "pi": pi, "S_sub": S_sub, "TPOS": TPOS, "NT": NT,
        "gidx16": gidx16, "dloc_c": dloc_c, "calls": calls,
        "sched_off": sched_off, "cons_pos": cons_pos,
    }


# ----------------------------------------------------------------------------
# Kernel builder
# ----------------------------------------------------------------------------

def _emit_hl(nc, tcx, emb_slab, shard_hl):
    """Split fp32 slab into bf16 hi|lo and write the interleaved shard rows."""
    with tcx.tile_pool(name="hl", bufs=1) as hp:
        hl = hp.tile([128, NBLK, 2 * D], bf16, tag="hl")
        tmp = hp.tile([128, NBLK, D], f32, tag="tmp")
        nc.vector.tensor_copy(out=hl[:, :, :D], in_=emb_slab[:])
        nc.vector.tensor_copy(out=tmp[:], in_=hl[:, :, :D])
        nc.vector.tensor_tensor(out=tmp[:], in0=emb_slab[:], in1=tmp[:],
                                op=Alu.subtract)
        nc.vector.tensor_copy(out=hl[:, :, D:], in_=tmp[:])
        nc.sync.dma_start(shard_hl[:, :], hl[:])


def _build(meta):
    S_sub = meta["S_sub"]
    TPOS = meta["TPOS"]
    NT = meta["NT"]
    calls = meta["calls"]
    cons_pos = meta["cons_pos"]

    nc = bacc.Bacc("TRN2", debug=False, num_swdge_queues=4)

    ne_tab = nc.dram_tensor("ne_tab", [128, D], f32, kind="ExternalInput")
    type_idx = nc.dram_tensor("type_idx", [128, SH // 16], i16, kind="ExternalInput")
    featT_in = nc.dram_tensor("featT", [DF + 1, SH], f32, kind="ExternalInput")
    wft_in = nc.dram_tensor("wft", [DF + 1, D], f32, kind="ExternalInput")
    wct_in = nc.dram_tensor("wct", [D, D], f32, kind="ExternalInput")
    cb_in = nc.dram_tensor("cb_col", [D, 1], f32, kind="ExternalInput")
    p_in = nc.dram_tensor("p_col", [D, 1], f32, kind="ExternalInput")
    ones_in = nc.dram_tensor("ones_col", [D, 1], f32, kind="ExternalInput")
    onesrow_in = nc.dram_tensor("ones_row", [1, 128], f32, kind="ExternalInput")
    ident_in = nc.dram_tensor("ident", [128, 128], f32, kind="ExternalInput")
    iota_in = nc.dram_tensor("iota_rep", [128, G_OH * 128], bf16, kind="ExternalInput")
    gidx_in = nc.dram_tensor("gidx16", [128, TPOS // 16], i16, kind="ExternalInput")
    dloc_in = nc.dram_tensor("dloc_c", [128, NT], f32, kind="ExternalInput")

    score_out = nc.dram_tensor("score", [128, NBLK], f32, kind="ExternalOutput")
    if PHASE in ("emb0", "agg", "emb1"):
        dbg_out = nc.dram_tensor("dbg", [SH, D], f32, kind="ExternalOutput")

    shard_hl = nc.dram_tensor("shard_hl", [SH, 2 * D], bf16)
    table_hl = nc.dram_tensor("table_hl", [NP, 2 * D], bf16, addr_space="Shared")

    import contextlib

    def allgather(tcx):
        with tcx.tile_critical():
            with contextlib.ExitStack() as st:
                cc = st.enter_context(nc.semaphore())
                nc.gpsimd.collective_compute(
                    "AllGather", Alu.bypass,
                    replica_groups=[list(range(W))],
                    ins=[shard_hl.ap().opt()],
                    outs=[table_hl.ap().opt()],
                ).then_inc(cc, 1)
                nc.gpsimd.wait_ge(cc, 1)

    # consumption-tile schedule per block: (first_tile_index, total_tiles)
    S_tot = S_sub.sum(1).astype(np.int64)
    blk_t0 = np.concatenate([[0], np.cumsum(S_tot)])[:-1]

    with tile.TileContext(nc) as tc:
        nc.gpsimd.load_library(library_config.mlp)
        with contextlib.ExitStack() as big:
            cpool = big.enter_context(tc.tile_pool(name="consts", bufs=1))
            slabs = big.enter_context(tc.tile_pool(name="slabs", bufs=1))

            wft_sb = cpool.tile([DF + 1, D], f32, tag="wft")
            nc.sync.dma_start(wft_sb[:], wft_in[:])
            wct_sb = cpool.tile([D, D], f32, tag="wct")
            nc.sync.dma_start(wct_sb[:], wct_in[:])
            cb_sb = cpool.tile([D, 1], f32, tag="cb")
            nc.sync.dma_start(cb_sb[:], cb_in[:])
            p_sb = cpool.tile([D, 1], f32, tag="p")
            nc.sync.dma_start(p_sb[:], p_in[:])
            ones_sb = cpool.tile([D, 1], f32, tag="ones")
            nc.sync.dma_start(ones_sb[:], ones_in[:])
            onesrow_sb = cpool.tile([1, 128], f32, tag="onesrow")
            nc.sync.dma_start(onesrow_sb[:], onesrow_in[:])
            ident_sb = cpool.tile([128, 128], f32, tag="ident")
            nc.sync.dma_start(ident_sb[:], ident_in[:])
            iota_sb = cpool.tile([128, G_OH * 128], bf16, tag="iota")
            nc.sync.dma_start(iota_sb[:], iota_in[:])
            dloc_sb = cpool.tile([128, NT], f32, tag="dloc")
            nc.sync.dma_start(dloc_sb[:], dloc_in[:])
            gidx_sb = cpool.tile([128, TPOS // 16], i16, tag="gidx")
            nc.sync.dma_start(gidx_sb[:], gidx_in[:])

            emb_slab = slabs.tile([128, NBLK, D], f32, tag="emb")
            agg_slab = slabs.tile([128, NBLK, D], f32, tag="agg")
            num_slab = slabs.tile([128, NBLK], f32, tag="num")
            nsq_slab = slabs.tile([128, NBLK], f32, tag="nsq")

            # ---------------- Phase 0: initial embeddings -------------------
            with tc.tile_pool(name="ph0", bufs=1) as ph0, \
                 tc.tile_pool(name="ph0ps", bufs=8, space="PSUM") as ph0ps:
                tix = ph0.tile([128, SH // 16], i16, tag="tix")
                nc.sync.dma_start(tix[:], type_idx[:])
                featT_sb = ph0.tile([DF + 1, SH], f32, tag="featT")
                nc.sync.dma_start(featT_sb[:], featT_in[:])
                tg_blks = [25, 25, 24, 24]
                tb = 0
                for q in range(4):
                    npos = tg_blks[q] * 128
                    nc.gpsimd.dma_gather(
                        emb_slab[:, tb:tb + tg_blks[q], :], ne_tab[:],
                        tix[:, tb * 8:(tb + tg_blks[q]) * 8],
                        npos, npos, D,
                        single_packet=False, queue_num=q)
                    tb += tg_blks[q]
                for b in range(NBLK):
                    ps = ph0ps.tile([128, D], f32, tag="p0")
                    nc.tensor.matmul(
                        ps[:], featT_sb[:, b * 128:(b + 1) * 128], wft_sb[:],
                        start=True, stop=True)
                    nc.vector.tensor_tensor(
                        out=emb_slab[:, b, :], in0=emb_slab[:, b, :], in1=ps[:],
                        op=Alu.add)
            _emit_hl(nc, tc, emb_slab, shard_hl)
            allgather(tc)

            if PHASE == "emb0":
                nc.sync.dma_start(dbg_out[:, :], emb_slab[:])

            # ---------------- rounds -----------------------------------------
            nrounds = 0 if PHASE == "emb0" else (1 if PHASE in ("agg", "emb1") else 2)
            for r in range(nrounds):
                last = r == nrounds - 1 and PHASE == "full"
                with tc.tile_pool(name=f"g{r}", bufs=int(os.environ.get("BASS_GNN_GBUFS", "10"))) as gpool, \
                     tc.tile_pool(name=f"oh{r}", bufs=4) as ohpool, \
                     tc.tile_pool(name=f"ps{r}", bufs=8, space="PSUM") as pspool:
                    call_tiles = {}
                    for (k, o, npos) in calls:
                        gt = gpool.tile([128, CALL // 128, 2 * D], bf16, tag="gbuf")
                        nc.gpsimd.dma_gather(
                            gt[:, :npos // 128, :],
                            table_hl[k * CH:(k + 1) * CH, :],
                            gidx_sb[:, o // 16:(o + npos) // 16],
                            npos, npos, 2 * D,
                            single_packet=False, queue_num=k)
                        call_tiles[o] = (gt, npos)
                    call_offs = sorted(call_tiles.keys())

                    import bisect

                    def tile_at(pos):
                        j = bisect.bisect_right(call_offs, pos) - 1
                        o = call_offs[j]
                        gt, npos = call_tiles[o]
                        assert o <= pos < o + npos
                        return gt[:, (pos - o) // 128, :]

                    oh_tiles = {}

                    def oh_at(t):
                        g0 = (t // G_OH) * G_OH
                        if g0 not in oh_tiles:
                            gsz = min(G_OH, NT - g0)
                            oh = ohpool.tile([128, G_OH, 128], bf16, tag="oh")
                            nc.vector.tensor_tensor(
                                out=oh[:, :gsz, :],
                                in0=iota_sb[:].rearrange(
                                    "p (g j) -> p g j", j=128)[:, :gsz, :],
                                in1=dloc_sb[:, g0:g0 + gsz].to_broadcast(
                                    [128, gsz, 128]),
                                op=Alu.is_equal)
                            oh_tiles[g0] = oh
                        return oh_tiles[g0][:, t - g0, :]

                    for b in range(NBLK):
                        stot = int(S_tot[b])
                        ps = pspool.tile([128, 2 * D], f32, tag="acc")
                        for j in range(stot):
                            t = int(blk_t0[b]) + j
                            nc.tensor.matmul(
                                ps[:], oh_at(t), tile_at(int(cons_pos[t])),
                                start=(j == 0), stop=(j == stot - 1))
                        nc.vector.reduce_sum(
                            out=agg_slab[:, b, :],
                            in_=ps[:].rearrange("p (h d) -> p d h", h=2),
                            axis=mybir.AxisListType.X)

                if PHASE == "agg" and r == 0:
                    nc.sync.dma_start(dbg_out[:, :], agg_slab[:])

                # x = emb + agg (in place into agg_slab)
                nc.vector.tensor_tensor(
                    out=agg_slab[:], in0=agg_slab[:], in1=emb_slab[:], op=Alu.add)

                # conv: relu(x @ W.T + b) via transposed layout; round 2 fuses
                # the cosine-score matmuls instead of transposing back.
                with tc.tile_pool(name=f"cv{r}", bufs=3) as cv, \
                     tc.tile_pool(name=f"cvps{r}", bufs=2, space="PSUM") as cvps:
                    for g0 in range(0, NBLK, 4):
                        blks = list(range(g0, min(g0 + 4, NBLK)))
                        wdt = 128 * len(blks)
                        pst = cvps.tile([D, 512], f32, tag="tr")
                        for j, b in enumerate(blks):
                            nc.tensor.transpose(
                                pst[:, j * 128:(j + 1) * 128],
                                agg_slab[:, b, :], ident_sb[:])
                        xT = cv.tile([D, 512], f32, tag="xT")
                        nc.vector.tensor_copy(out=xT[:, :wdt], in_=pst[:, :wdt])
                        zps = cvps.tile([D, 512], f32, tag="z")
                        nc.tensor.matmul(
                            zps[:, :wdt], wct_sb[:], xT[:, :wdt],
                            start=True, stop=True)
                        zr = cv.tile([D, 512], f32, tag="zr")
                        nc.vector.tensor_scalar(
                            zr[:, :wdt], zps[:, :wdt], cb_sb[:, :1], 0.0,
                            Alu.add, Alu.max)
                        if not last:
                            psb = cvps.tile([128, 4 * D], f32, tag="trb")
                            for j, b in enumerate(blks):
                                nc.tensor.transpose(
                                    psb[:, j * D:(j + 1) * D],
                                    zr[:, j * 128:(j + 1) * 128],
                                    ident_sb[:D, :D])
                            nc.vector.tensor_copy(
                                out=emb_slab[:, g0:g0 + len(blks), :],
                                in_=psb[:, :len(blks) * D])
                        else:
                            sq = cv.tile([D, 512], f32, tag="sq")
                            nc.vector.tensor_tensor(
                                out=sq[:, :wdt], in0=zr[:, :wdt],
                                in1=zr[:, :wdt], op=Alu.mult)
                            nps = cvps.tile([128, 4], f32, tag="nps")
                            qps = cvps.tile([128, 4], f32, tag="qps")
                            for j, b in enumerate(blks):
                                nc.tensor.matmul(
                                    nps[:, j:j + 1],
                                    zr[:, j * 128:(j + 1) * 128], p_sb[:],
                                    start=True, stop=True)
                                nc.tensor.matmul(
                                    qps[:, j:j + 1],
                                    sq[:, j * 128:(j + 1) * 128], ones_sb[:],
                                    start=True, stop=True)
                            nc.vector.tensor_copy(
                                out=num_slab[:, g0:g0 + len(blks)],
                                in_=nps[:, :len(blks)])
                            nc.vector.tensor_copy(
                                out=nsq_slab[:, g0:g0 + len(blks)],
                                in_=qps[:, :len(blks)])
                if not last:
                    _emit_hl(nc, tc, emb_slab, shard_hl)
                    if r == 0 and nrounds == 2:
                        allgather(tc)
                    if PHASE == "emb1" and r == 0:
                        nc.sync.dma_start(dbg_out[:, :], emb_slab[:])

            # ---------------- cosine scores ----------------------------------
            if PHASE == "full":
                with tc.tile_pool(name="cos", bufs=1) as cos, \
                     tc.tile_pool(name="cosps", bufs=2, space="PSUM") as cosps:
                    # pnorm = max(||p||, eps) replicated to [128,1]
                    psq = cos.tile([D, 1], f32, tag="psq")
                    nc.vector.tensor_tensor(
                        out=psq[:], in0=p_sb[:], in1=p_sb[:], op=Alu.mult)
                    pn_ps = cosps.tile([1, 1], f32, tag="pn")
                    nc.tensor.matmul(pn_ps[:], psq[:], ones_sb[:],
                                     start=True, stop=True)
                    pn_sb = cos.tile([1, 1], f32, tag="pnsb")
                    nc.scalar.activation(pn_sb[:], pn_ps[:], Act.Sqrt)
                    nc.vector.tensor_scalar(
                        pn_sb[:], pn_sb[:], EPS, None, Alu.max)
                    pnr_ps = cosps.tile([128, 1], f32, tag="pnr")
                    nc.tensor.matmul(pnr_ps[:], onesrow_sb[:], pn_sb[:],
                                     start=True, stop=True)
                    pnrep = cos.tile([128, 1], f32, tag="pnrep")
                    nc.vector.tensor_copy(out=pnrep[:], in_=pnr_ps[:])

                    norm = cos.tile([128, NBLK], f32, tag="norm")
                    nc.scalar.activation(norm[:], nsq_slab[:], Act.Sqrt)
                    nc.vector.tensor_scalar(
                        norm[:], norm[:], EPS, None, Alu.max)
                    nc.vector.tensor_scalar(
                        norm[:], norm[:], pnrep[:, :1], None, Alu.mult)
                    nc.vector.reciprocal(norm[:], norm[:])
                    nc.vector.tensor_tensor(
                        out=num_slab[:], in0=num_slab[:], in1=norm[:],
                        op=Alu.mult)
                    nc.sync.dma_start(score_out[:, :], num_slab[:])
            else:
                nc.vector.memset(num_slab[:, :1], 0.0)
                nc.sync.dma_start(score_out[:, :1], num_slab[:, :1])

    nc.compile()
    return nc


# ----------------------------------------------------------------------------
# Public entry
# ----------------------------------------------------------------------------

_cache = {}


def kernel(nodes, edges, features, node_emb, feat_W, feat_b,
           conv1_W, conv1_b, pattern_emb, pattern_id):
    import ml_dtypes

    nodes = np.asarray(nodes)
    edges = np.asarray(edges)
    features = np.asarray(features, np.float32)
    node_emb = np.asarray(node_emb, np.float32)
    feat_W = np.asarray(feat_W, np.float32)
    feat_b = np.asarray(feat_b, np.float32)
    conv1_W = np.asarray(conv1_W, np.float32)
    conv1_b = np.asarray(conv1_b, np.float32)
    pattern_emb = np.asarray(pattern_emb, np.float32)
    pid = int(np.asarray(pattern_id))

    meta = _prep(nodes, edges)
    pi = meta["pi"]

    if "k" not in _cache:
        _cache["k"] = _build(meta)
    nc = _cache["k"]

    types_p = np.zeros(NP, np.int64)
    types_p[pi[:N]] = nodes.astype(np.int64)
    feat_p = np.zeros((NP, DF), np.float32)
    feat_p[pi[:N]] = features

    ne_tab = np.zeros((128, D), np.float32)
    ne_tab[:VOCAB] = node_emb
    wft = np.concatenate([feat_W.T, feat_b[None, :]], 0).astype(np.float32)
    wct = conv1_W.T.copy()
    cb_col = conv1_b.reshape(D, 1).copy()
    p_col = pattern_emb[pid].reshape(D, 1).copy()
    ones_col = np.ones((D, 1), np.float32)
    ones_row = np.ones((1, 128), np.float32)
    ident = np.eye(128, dtype=np.float32)
    iota_rep = np.broadcast_to(np.arange(128).astype(ml_dtypes.bfloat16),
                               (128, G_OH, 128)).reshape(128, G_OH * 128).copy()

    in_maps = []
    for c in range(W):
        rows = slice(c * SH, (c + 1) * SH)
        # table rows are p-major (row = p*NBLK + b) but dma_gather writes
        # stream position i to slab [i%128, i//128] = [p, b] -> feed the type
        # stream in (b, p) order so row p*NBLK+b lands at [p, b].
        tv = types_p[rows].reshape(128, NBLK).T.ravel()
        t16 = tv.reshape(-1, 16).T.astype(np.int16)
        t16 = np.tile(t16, (8, 1))
        fv = feat_p[rows].reshape(128, NBLK, DF).transpose(1, 0, 2)
        featT_c = fv.reshape(SH, DF).T
        featT_c = np.concatenate([featT_c, np.ones((1, SH), np.float32)], 0)
        in_maps.append({
            "ne_tab": ne_tab,
            "type_idx": t16,
            "featT": np.ascontiguousarray(featT_c),
            "wft": wft, "wct": wct,
            "cb_col": cb_col, "p_col": p_col, "ones_col": ones_col,
            "ones_row": ones_row, "ident": ident, "iota_rep": iota_rep,
            "gidx16": meta["gidx16"][c],
            "dloc_c": meta["dloc_c"][c],
        })

    tdir = os.environ.get("BASS_GNN_TRACE_DIR") or None
    res = run_bass_kernel_spmd(nc, in_maps, core_ids=list(range(W)),
                               trace=TRACE, tmpdir=tdir)
    kernel.last_results = res

    if PHASE != "full":
        dump = np.concatenate([res.results[c]["dbg"] for c in range(W)], 0)
        return dump

    out_p = np.empty(NP, np.float32)
    for c in range(W):
        s = res.results[c]["score"]
        out_p[c * SH:(c + 1) * SH] = s.reshape(SH)
    return out_p[pi[:N]]


# revision 24
# speedup vs baseline: 1.0352x; 1.0352x over previous
"""GNN message-passing kernel for Trainium2 (8 NeuronCores, Bass/Tile).

Implements:
  embeds = node_emb[nodes] + features @ feat_W.T + feat_b
  2 x { agg[dst] += embeds[src];  embeds = relu((embeds+agg) @ conv1_W.T + conv1_b) }
  out = cosine(embeds, pattern_emb[pattern_id])

Distribution: nodes are permuted (in-degree balanced) and sharded over 8 cores
(12544 nodes each).  Edges are partitioned by dst owner; messages are fetched
with GPSIMD dma_gather (4 SWDGE queues, one per table chunk) from a replicated
bf16 hi|lo DRAM table rebuilt each round with an ncfw AllGather.  Per 128-dst
block the segment-sum is a one-hot matmul accumulated in PSUM.
"""

import os
import sys

sys.path.insert(0, "/opt/trn_rl_repo")

import numpy as np

import concourse.bass as bass
import concourse.bacc as bacc
import concourse.mybir as mybir
import concourse.tile as tile
from concourse import library_config
from concourse.bass_utils import run_bass_kernel_spmd

N = 100000
NP = 100352          # padded: 8 * 12544
W = 8                # cores
SH = NP // W         # 12544 nodes per core
NBLK = SH // 128     # 98 blocks per core
NBG = NP // 128      # 784 global blocks
NCHUNK = 4
CH = NP // NCHUNK    # 25088 rows per gather chunk (int16-safe)
D = 64
DF = 10
VOCAB = 100
EPS = 1e-8
CALL = int(os.environ.get("BASS_GNN_CALL", "768"))          # gather positions per dma_gather call
G_OH = 16            # one-hot tiles built per DVE op
PHASE = os.environ.get("BASS_GNN_PHASE", "full")
TRACE = os.environ.get("BASS_GNN_TRACE", "0") == "1"

f32 = mybir.dt.float32
bf16 = mybir.dt.bfloat16
i16 = mybir.dt.int16
Alu = mybir.AluOpType
Act = mybir.ActivationFunctionType


# ----------------------------------------------------------------------------
# Host-side preprocessing: permutation, block assignment, edge streams.
# ----------------------------------------------------------------------------

def _prep(nodes, edges):
    src = edges[:, 0].astype(np.int64)
    dst = edges[:, 1].astype(np.int64)

    indeg = np.zeros(NP, np.int64)
    np.add.at(indeg, dst, 1)
    order = np.argsort(-indeg, kind="stable")
    # round-robin by in-degree over 784 global blocks -> balanced block loads
    gblock = np.empty(NP, np.int64)
    gslot = np.empty(NP, np.int64)
    gblock[order] = np.arange(NP) % NBG
    gslot[order] = np.arange(NP) // NBG

    # block -> core is FIXED (g % W) so each node's gather chunk (= its core
    # pair) is known exactly; then align per-(block,chunk) tile needs across
    # cores by sorting each core's blocks by their tile-need pattern.
    core_of_gblock = np.arange(NBG) % W
    chunk_n = core_of_gblock[gblock] // 2       # chunk of node as gather src
    E0 = np.zeros((NBG, NCHUNK), np.int64)
    np.add.at(E0, (gblock[dst], chunk_n[src]), 1)
    t0 = np.ceil(E0 / 128).astype(np.int64)
    key = (t0 * (5 ** np.arange(NCHUNK))[::-1]).sum(1) * 1000 + E0.sum(1) // 16
    lslot_of_gblock = np.empty(NBG, np.int64)
    for c in range(W):
        mine = np.where(core_of_gblock == c)[0]
        order_c = mine[np.argsort(key[mine], kind="stable")]
        lslot_of_gblock[order_c] = np.arange(len(order_c))

    # final permutation: pi(n) = core*SH + slot_within_block*NBLK + local_block
    # (p-major layout so SBUF slab [128, 98, 64] maps contiguously to DRAM rows)
    c_n = core_of_gblock[gblock]
    b_n = lslot_of_gblock[gblock]
    p_n = gslot
    pi = c_n * SH + p_n * NBLK + b_n

    # exact per-(core, local block, chunk) edge counts under final assignment
    e_dc = c_n[dst]
    e_db = b_n[dst]
    e_dp = p_n[dst]
    e_chunk = pi[src] // CH
    Ec = np.zeros((W, NBLK, NCHUNK), np.int64)
    np.add.at(Ec, (e_dc, e_db, e_chunk), 1)
    S_sub = np.ceil(Ec / 128).astype(np.int64).max(0)  # [NBLK, NCHUNK]
    S_sub = np.maximum(S_sub, 1)   # keep schedule uniform & non-degenerate
    assert S_sub.max() <= CALL // 128

    # edge order: (core, block, chunk, src)
    ek = np.lexsort((pi[src], e_chunk, e_db, e_dc))
    src_pi = pi[src][ek]
    dloc_e = e_dp[ek]
    key_cbk = (e_dc[ek] * NBLK + e_db[ek]) * NCHUNK + e_chunk[ek]
    counts = np.zeros(W * NBLK * NCHUNK + 1, np.int64)
    np.add.at(counts, key_cbk + 1, 1)
    starts = np.cumsum(counts)

    # gather-stream offsets: chunk-major, then block, then tile
    TPOS = int(S_sub.sum() * 128)
    NT = TPOS // 128
    sched_off = np.zeros((NCHUNK, NBLK), np.int64)
    off = 0
    for k in range(NCHUNK):
        for b in range(NBLK):
            sched_off[k, b] = off
            off += int(S_sub[b, k]) * 128
    assert off == TPOS

    gidx = np.zeros((W, TPOS), np.int32)
    dloc = np.full((W, TPOS), -1.0, np.float32)   # stream order
    for c in range(W):
        for k in range(NCHUNK):
            for b in range(NBLK):
                i0 = starts[(c * NBLK + b) * NCHUNK + k]
                i1 = starts[(c * NBLK + b) * NCHUNK + k + 1]
                n = i1 - i0
                o = sched_off[k, b]
                assert n <= S_sub[b, k] * 128
                gidx[c, o:o + n] = src_pi[i0:i1] - k * CH
                dloc[c, o:o + n] = dloc_e[i0:i1]
    assert gidx.min() >= 0 and gidx.max() < CH

    # wrapped-16 int16 idx layout (stream order)
    gidx16 = np.zeros((W, 128, TPOS // 16), np.int16)
    for c in range(W):
        g = gidx[c].reshape(-1, 16).T.astype(np.int16)
        gidx16[c] = np.tile(g, (8, 1))

    # consumption order: block-major (b, k, s) -> stream position of each tile
    cons_pos = []
    for b in range(NBLK):
        for k in range(NCHUNK):
            for s in range(int(S_sub[b, k])):
                cons_pos.append(int(sched_off[k, b]) + s * 128)
    cons_pos = np.array(cons_pos, np.int64)
    assert len(cons_pos) == NT
    # dloc in consumption-tile-column layout [128, NT]
    dloc_s = dloc.reshape(W, NT, 128)        # stream tile t at rows [t]
    stream_tile = cons_pos // 128
    dloc_c = dloc_s[:, stream_tile, :].transpose(0, 2, 1).astype(np.float32).copy()

    # per-chunk call plan, issued round-robin across chunks
    ncalls_k = []
    base_k = []
    for k in range(NCHUNK):
        npos = int(S_sub[:, k].sum() * 128)
        base_k.append(int(sched_off[k, 0]))
        ncalls_k.append((npos + CALL - 1) // CALL)
    calls = []   # (k, stream_off, npos) in issue order
    for ci in range(max(ncalls_k)):
        for k in range(NCHUNK):
            if ci < ncalls_k[k]:
                npos_k = int(S_sub[:, k].sum() * 128)
                o = base_k[k] + ci * CALL
                n = min(CALL, base_k[k] + npos_k - o)
                calls.append((k, o, n))

    return {
        "pi": pi, "S_sub": S_sub, "TPOS": TPOS, "NT": NT,
        "gidx16": gidx16, "dloc_c": dloc_c, "calls": calls,
        "sched_off": sched_off, "cons_pos": cons_pos,
    }


# ----------------------------------------------------------------------------
# Kernel builder
# ----------------------------------------------------------------------------

def _emit_hl(nc, tcx, emb_slab, shard_hl):
    """Split fp32 slab into bf16 hi|lo and write the interleaved shard rows."""
    with tcx.tile_pool(name="hl", bufs=1) as hp:
        hl = hp.tile([128, NBLK, 2 * D], bf16, tag="hl")
        tmp = hp.tile([128, NBLK, D], f32, tag="tmp")
        nc.vector.tensor_copy(out=hl[:, :, :D], in_=emb_slab[:])
        nc.vector.tensor_copy(out=tmp[:], in_=hl[:, :, :D])
        nc.vector.tensor_tensor(out=tmp[:], in0=emb_slab[:], in1=tmp[:],
                                op=Alu.subtract)
        nc.vector.tensor_copy(out=hl[:, :, D:], in_=tmp[:])
        nc.sync.dma_start(shard_hl[:, :], hl[:])


def _build(meta):
    S_sub = meta["S_sub"]
    TPOS = meta["TPOS"]
    NT = meta["NT"]
    calls = meta["calls"]
    cons_pos = meta["cons_pos"]

    nc = bacc.Bacc("TRN2", debug=False, num_swdge_queues=4)

    ne_tab = nc.dram_tensor("ne_tab", [128, D], f32, kind="ExternalInput")
    type_idx = nc.dram_tensor("type_idx", [128, SH // 16], i16, kind="ExternalInput")
    featT_in = nc.dram_tensor("featT", [DF + 1, SH], f32, kind="ExternalInput")
    wft_in = nc.dram_tensor("wft", [DF + 1, D], f32, kind="ExternalInput")
    wct_in = nc.dram_tensor("wct", [D, D], f32, kind="ExternalInput")
    cb_in = nc.dram_tensor("cb_col", [D, 1], f32, kind="ExternalInput")
    p_in = nc.dram_tensor("p_col", [D, 1], f32, kind="ExternalInput")
    ones_in = nc.dram_tensor("ones_col", [D, 1], f32, kind="ExternalInput")
    onesrow_in = nc.dram_tensor("ones_row", [1, 128], f32, kind="ExternalInput")
    ident_in = nc.dram_tensor("ident", [128, 128], f32, kind="ExternalInput")
    iota_in = nc.dram_tensor("iota_rep", [128, G_OH * 128], bf16, kind="ExternalInput")
    gidx_in = nc.dram_tensor("gidx16", [128, TPOS // 16], i16, kind="ExternalInput")
    dloc_in = nc.dram_tensor("dloc_c", [128, NT], f32, kind="ExternalInput")

    score_out = nc.dram_tensor("score", [128, NBLK], f32, kind="ExternalOutput")
    if PHASE in ("emb0", "agg", "emb1"):
        dbg_out = nc.dram_tensor("dbg", [SH, D], f32, kind="ExternalOutput")

    shard_hl = nc.dram_tensor("shard_hl", [SH, 2 * D], bf16)
    table_hl = nc.dram_tensor("table_hl", [NP, 2 * D], bf16, addr_space="Shared")

    import contextlib

    def allgather(tcx):
        with tcx.tile_critical():
            with contextlib.ExitStack() as st:
                cc = st.enter_context(nc.semaphore())
                nc.gpsimd.collective_compute(
                    "AllGather", Alu.bypass,
                    replica_groups=[list(range(W))],
                    ins=[shard_hl.ap().opt()],
                    outs=[table_hl.ap().opt()],
                ).then_inc(cc, 1)
                nc.gpsimd.wait_ge(cc, 1)

    # consumption-tile schedule per block: (first_tile_index, total_tiles)
    S_tot = S_sub.sum(1).astype(np.int64)
    blk_t0 = np.concatenate([[0], np.cumsum(S_tot)])[:-1]

    with tile.TileContext(nc) as tc:
        nc.gpsimd.load_library(library_config.mlp)
        with contextlib.ExitStack() as big:
            cpool = big.enter_context(tc.tile_pool(name="consts", bufs=1))
            slabs = big.enter_context(tc.tile_pool(name="slabs", bufs=1))

            wft_sb = cpool.tile([DF + 1, D], f32, tag="wft")
            nc.sync.dma_start(wft_sb[:], wft_in[:])
            wct_sb = cpool.tile([D, D], f32, tag="wct")
            nc.sync.dma_start(wct_sb[:], wct_in[:])
            cb_sb = cpool.tile([D, 1], f32, tag="cb")
            nc.sync.dma_start(cb_sb[:], cb_in[:])
            p_sb = cpool.tile([D, 1], f32, tag="p")
            nc.sync.dma_start(p_sb[:], p_in[:])
            ones_sb = cpool.tile([D, 1], f32, tag="ones")
            nc.sync.dma_start(ones_sb[:], ones_in[:])
            onesrow_sb = cpool.tile([1, 128], f32, tag="onesrow")
            nc.sync.dma_start(onesrow_sb[:], onesrow_in[:])
            ident_sb = cpool.tile([128, 128], f32, tag="ident")
            nc.sync.dma_start(ident_sb[:], ident_in[:])
            iota_sb = cpool.tile([128, G_OH * 128], bf16, tag="iota")
            nc.sync.dma_start(iota_sb[:], iota_in[:])
            dloc_sb = cpool.tile([128, NT], f32, tag="dloc")
            nc.sync.dma_start(dloc_sb[:], dloc_in[:])
            gidx_sb = cpool.tile([128, TPOS // 16], i16, tag="gidx")
            nc.sync.dma_start(gidx_sb[:], gidx_in[:])

            emb_slab = slabs.tile([128, NBLK, D], f32, tag="emb")
            agg_slab = slabs.tile([128, NBLK, D], f32, tag="agg")
            num_slab = slabs.tile([128, NBLK], f32, tag="num")
            nsq_slab = slabs.tile([128, NBLK], f32, tag="nsq")

            # ---------------- Phase 0: initial embeddings -------------------
            with tc.tile_pool(name="ph0", bufs=1) as ph0, \
                 tc.tile_pool(name="ph0ps", bufs=8, space="PSUM") as ph0ps:
                tix = ph0.tile([128, SH // 16], i16, tag="tix")
                nc.sync.dma_start(tix[:], type_idx[:])
                featT_sb = ph0.tile([DF + 1, SH], f32, tag="featT")
                nc.sync.dma_start(featT_sb[:], featT_in[:])
                tg_blks = [25, 25, 24, 24]
                tb = 0
                for q in range(4):
                    npos = tg_blks[q] * 128
                    nc.gpsimd.dma_gather(
                        emb_slab[:, tb:tb + tg_blks[q], :], ne_tab[:],
                        tix[:, tb * 8:(tb + tg_blks[q]) * 8],
                        npos, npos, D,
                        single_packet=False, queue_num=q)
                    tb += tg_blks[q]
                for b in range(NBLK):
                    ps = ph0ps.tile([128, D], f32, tag="p0")
                    nc.tensor.matmul(
                        ps[:], featT_sb[:, b * 128:(b + 1) * 128], wft_sb[:],
                        start=True, stop=True)
                    nc.vector.tensor_tensor(
                        out=emb_slab[:, b, :], in0=emb_slab[:, b, :], in1=ps[:],
                        op=Alu.add)
            _emit_hl(nc, tc, emb_slab, shard_hl)
            allgather(tc)

            if PHASE == "emb0":
                nc.sync.dma_start(dbg_out[:, :], emb_slab[:])

            # ---------------- rounds -----------------------------------------
            nrounds = 0 if PHASE == "emb0" else (1 if PHASE in ("agg", "emb1") else 2)
            for r in range(nrounds):
                last = r == nrounds - 1 and PHASE == "full"
                with tc.tile_pool(name=f"g{r}", bufs=int(os.environ.get("BASS_GNN_GBUFS", "32"))) as gpool, \
                     tc.tile_pool(name=f"oh{r}", bufs=4) as ohpool, \
                     tc.tile_pool(name=f"ps{r}", bufs=8, space="PSUM") as pspool:
                    call_tiles = {}
                    for (k, o, npos) in calls:
                        gt = gpool.tile([128, CALL // 128, 2 * D], bf16, tag="gbuf")
                        nc.gpsimd.dma_gather(
                            gt[:, :npos // 128, :],
                            table_hl[k * CH:(k + 1) * CH, :],
                            gidx_sb[:, o // 16:(o + npos) // 16],
                            npos, npos, 2 * D,
                            single_packet=False, queue_num=k)
                        call_tiles[o] = (gt, npos)
                    call_offs = sorted(call_tiles.keys())

                    import bisect

                    def tile_at(pos):
                        j = bisect.bisect_right(call_offs, pos) - 1
                        o = call_offs[j]
                        gt, npos = call_tiles[o]
                        assert o <= pos < o + npos
                        return gt[:, (pos - o) // 128, :]

                    oh_tiles = {}

                    def oh_at(t):
                        g0 = (t // G_OH) * G_OH
                        if g0 not in oh_tiles:
                            gsz = min(G_OH, NT - g0)
                            oh = ohpool.tile([128, G_OH, 128], bf16, tag="oh")
                            nc.vector.tensor_tensor(
                                out=oh[:, :gsz, :],
                                in0=iota_sb[:].rearrange(
                                    "p (g j) -> p g j", j=128)[:, :gsz, :],
                                in1=dloc_sb[:, g0:g0 + gsz].to_broadcast(
                                    [128, gsz, 128]),
                                op=Alu.is_equal)
                            oh_tiles[g0] = oh
                        return oh_tiles[g0][:, t - g0, :]

                    for b in range(NBLK):
                        stot = int(S_tot[b])
                        ps = pspool.tile([128, 2 * D], f32, tag="acc")
                        for j in range(stot):
                            t = int(blk_t0[b]) + j
                            nc.tensor.matmul(
                                ps[:], oh_at(t), tile_at(int(cons_pos[t])),
                                start=(j == 0), stop=(j == stot - 1))
                        nc.vector.reduce_sum(
                            out=agg_slab[:, b, :],
                            in_=ps[:].rearrange("p (h d) -> p d h", h=2),
                            axis=mybir.AxisListType.X)

                if PHASE == "agg" and r == 0:
                    nc.sync.dma_start(dbg_out[:, :], agg_slab[:])

                # x = emb + agg (in place into agg_slab)
                nc.vector.tensor_tensor(
                    out=agg_slab[:], in0=agg_slab[:], in1=emb_slab[:], op=Alu.add)

                # conv: relu(x @ W.T + b) via transposed layout; round 2 fuses
                # the cosine-score matmuls instead of transposing back.
                with tc.tile_pool(name=f"cv{r}", bufs=3) as cv, \
                     tc.tile_pool(name=f"cvps{r}", bufs=2, space="PSUM") as cvps:
                    for g0 in range(0, NBLK, 4):
                        blks = list(range(g0, min(g0 + 4, NBLK)))
                        wdt = 128 * len(blks)
                        pst = cvps.tile([D, 512], f32, tag="tr")
                        for j, b in enumerate(blks):
                            nc.tensor.transpose(
                                pst[:, j * 128:(j + 1) * 128],
                                agg_slab[:, b, :], ident_sb[:])
                        xT = cv.tile([D, 512], f32, tag="xT")
                        nc.vector.tensor_copy(out=xT[:, :wdt], in_=pst[:, :wdt])
                        zps = cvps.tile([D, 512], f32, tag="z")
                        nc.tensor.matmul(
                            zps[:, :wdt], wct_sb[:], xT[:, :wdt],
                            start=True, stop=True)
                        zr = cv.tile([D, 512], f32, tag="zr")
                        nc.vector.tensor_scalar(
                            zr[:, :wdt], zps[:, :wdt], cb_sb[:, :1], 0.0,
                            Alu.add, Alu.max)
                        if not last:
                            psb = cvps.tile([128, 4 * D], f32, tag="trb")
                            for j, b in enumerate(blks):
                                nc.tensor.transpose(
                                    psb[:, j * D:(j + 1) * D],
                                    zr[:, j * 128:(j + 1) * 128],
                                    ident_sb[:D, :D])
                            nc.vector.tensor_copy(
                                out=emb_slab[:, g0:g0 + len(blks), :],
                                in_=psb[:, :len(blks) * D])
                        else:
                            sq = cv.tile([D, 512], f32, tag="sq")
                            nc.vector.tensor_tensor(
                                out=sq[:, :wdt], in0=zr[:, :wdt],
                                in1=zr[:, :wdt], op=Alu.mult)
                            nps = cvps.tile([128, 4], f32, tag="nps")
                            qps = cvps.tile([128, 4], f32, tag="qps")
                            for j, b in enumerate(blks):
                                nc.tensor.matmul(
                                    nps[:, j:j + 1],
                                    zr[:, j * 128:(j + 1) * 128], p_sb[:],
                                    start=True, stop=True)
                                nc.tensor.matmul(
                                    qps[:, j:j + 1],
                                    sq[:, j * 128:(j + 1) * 128], ones_sb[:],
                                    start=True, stop=True)
                            nc.vector.tensor_copy(
                                out=num_slab[:, g0:g0 + len(blks)],
                                in_=nps[:, :len(blks)])
                            nc.vector.tensor_copy(
                                out=nsq_slab[:, g0:g0 + len(blks)],
                                in_=qps[:, :len(blks)])
                if not last:
                    _emit_hl(nc, tc, emb_slab, shard_hl)
                    if r == 0 and nrounds == 2:
                        allgather(tc)
                    if PHASE == "emb1" and r == 0:
                        nc.sync.dma_start(dbg_out[:, :], emb_slab[:])

            # ---------------- cosine scores ----------------------------------
            if PHASE == "full":
                with tc.tile_pool(name="cos", bufs=1) as cos, \
                     tc.tile_pool(name="cosps", bufs=2, space="PSUM") as cosps:
                    # pnorm = max(||p||, eps) replicated to [128,1]
                    psq = cos.tile([D, 1], f32, tag="psq")
                    nc.vector.tensor_tensor(
                        out=psq[:], in0=p_sb[:], in1=p_sb[:], op=Alu.mult)
                    pn_ps = cosps.tile([1, 1], f32, tag="pn")
                    nc.tensor.matmul(pn_ps[:], psq[:], ones_sb[:],
                                     start=True, stop=True)
                    pn_sb = cos.tile([1, 1], f32, tag="pnsb")
                    nc.scalar.activation(pn_sb[:], pn_ps[:], Act.Sqrt)
                    nc.vector.tensor_scalar(
                        pn_sb[:], pn_sb[:], EPS, None, Alu.max)
                    pnr_ps = cosps.tile([128, 1], f32, tag="pnr")
                    nc.tensor.matmul(pnr_ps[:], onesrow_sb[:], pn_sb[:],
                                     start=True, stop=True)
                    pnrep = cos.tile([128, 1], f32, tag="pnrep")
                    nc.vector.tensor_copy(out=pnrep[:], in_=pnr_ps[:])

                    norm = cos.tile([128, NBLK], f32, tag="norm")
                    nc.scalar.activation(norm[:], nsq_slab[:], Act.Sqrt)
                    nc.vector.tensor_scalar(
                        norm[:], norm[:], EPS, None, Alu.max)
                    nc.vector.tensor_scalar(
                        norm[:], norm[:], pnrep[:, :1], None, Alu.mult)
                    nc.vector.reciprocal(norm[:], norm[:])
                    nc.vector.tensor_tensor(
                        out=num_slab[:], in0=num_slab[:], in1=norm[:],
                        op=Alu.mult)
                    nc.sync.dma_start(score_out[:, :], num_slab[:])
            else:
                nc.vector.memset(num_slab[:, :1], 0.0)
                nc.sync.dma_start(score_out[:, :1], num_slab[:, :1])

    nc.compile()
    return nc


# ----------------------------------------------------------------------------
# Public entry
# ----------------------------------------------------------------------------

_cache = {}


def kernel(nodes, edges, features, node_emb, feat_W, feat_b,
           conv1_W, conv1_b, pattern_emb, pattern_id):
    import ml_dtypes

    nodes = np.asarray(nodes)
    edges = np.asarray(edges)
    features = np.asarray(features, np.float32)
    node_emb = np.asarray(node_emb, np.float32)
    feat_W = np.asarray(feat_W, np.float32)
    feat_b = np.asarray(feat_b, np.float32)
    conv1_W = np.asarray(conv1_W, np.float32)
    conv1_b = np.asarray(conv1_b, np.float32)
    pattern_emb = np.asarray(pattern_emb, np.float32)
    pid = int(np.asarray(pattern_id))

    meta = _prep(nodes, edges)
    pi = meta["pi"]

    if "k" not in _cache:
        _cache["k"] = _build(meta)
    nc = _cache["k"]

    types_p = np.zeros(NP, np.int64)
    types_p[pi[:N]] = nodes.astype(np.int64)
    feat_p = np.zeros((NP, DF), np.float32)
    feat_p[pi[:N]] = features

    ne_tab = np.zeros((128, D), np.float32)
    ne_tab[:VOCAB] = node_emb
    wft = np.concatenate([feat_W.T, feat_b[None, :]], 0).astype(np.float32)
    wct = conv1_W.T.copy()
    cb_col = conv1_b.reshape(D, 1).copy()
    p_col = pattern_emb[pid].reshape(D, 1).copy()
    ones_col = np.ones((D, 1), np.float32)
    ones_row = np.ones((1, 128), np.float32)
    ident = np.eye(128, dtype=np.float32)
    iota_rep = np.broadcast_to(np.arange(128).astype(ml_dtypes.bfloat16),
                               (128, G_OH, 128)).reshape(128, G_OH * 128).copy()

    in_maps = []
    for c in range(W):
        rows = slice(c * SH, (c + 1) * SH)
        # table rows are p-major (row = p*NBLK + b) but dma_gather writes
        # stream position i to slab [i%128, i//128] = [p, b] -> feed the type
        # stream in (b, p) order so row p*NBLK+b lands at [p, b].
        tv = types_p[rows].reshape(128, NBLK).T.ravel()
        t16 = tv.reshape(-1, 16).T.astype(np.int16)
        t16 = np.tile(t16, (8, 1))
        fv = feat_p[rows].reshape(128, NBLK, DF).transpose(1, 0, 2)
        featT_c = fv.reshape(SH, DF).T
        featT_c = np.concatenate([featT_c, np.ones((1, SH), np.float32)], 0)
        in_maps.append({
            "ne_tab": ne_tab,
            "type_idx": t16,
            "featT": np.ascontiguousarray(featT_c),
            "wft": wft, "wct": wct,
            "cb_col": cb_col, "p_col": p_col, "ones_col": ones_col,
            "ones_row": ones_row, "ident": ident, "iota_rep": iota_rep,
            "gidx16": meta["gidx16"][c],
            "dloc_c": meta["dloc_c"][c],
        })

    tdir = os.environ.get("BASS_GNN_TRACE_DIR") or None
    res = run_bass_kernel_spmd(nc, in_maps, core_ids=list(range(W)),
                               trace=TRACE, tmpdir=tdir)
    kernel.last_results = res

    if PHASE != "full":
        dump = np.concatenate([res.results[c]["dbg"] for c in range(W)], 0)
        return dump

    out_p = np.empty(NP, np.float32)
    for c in range(W):
        s = res.results[c]["score"]
        out_p[c * SH:(c + 1) * SH] = s.reshape(SH)
    return out_p[pi[:N]]


# revision 26
# speedup vs baseline: 1.0778x; 1.0412x over previous
"""GNN message-passing kernel for Trainium2 (8 NeuronCores, Bass/Tile).

Implements:
  embeds = node_emb[nodes] + features @ feat_W.T + feat_b
  2 x { agg[dst] += embeds[src];  embeds = relu((embeds+agg) @ conv1_W.T + conv1_b) }
  out = cosine(embeds, pattern_emb[pattern_id])

Distribution: nodes are permuted (in-degree balanced) and sharded over 8 cores
(12544 nodes each).  Edges are partitioned by dst owner; messages are fetched
with GPSIMD dma_gather (4 SWDGE queues, one per table chunk) from a replicated
bf16 hi|lo DRAM table rebuilt each round with an ncfw AllGather.  Per 128-dst
block the segment-sum is a one-hot matmul accumulated in PSUM.
"""

import os
import sys

sys.path.insert(0, "/opt/trn_rl_repo")

import numpy as np

import concourse.bass as bass
import concourse.bacc as bacc
import concourse.mybir as mybir
import concourse.tile as tile
from concourse import library_config
from concourse.bass_utils import run_bass_kernel_spmd

N = 100000
NP = 100352          # padded: 8 * 12544
W = 8                # cores
SH = NP // W         # 12544 nodes per core
NBLK = SH // 128     # 98 blocks per core
NBG = NP // 128      # 784 global blocks
NCHUNK = 4
CH = NP // NCHUNK    # 25088 rows per gather chunk (int16-safe)
D = 64
DF = 10
VOCAB = 100
EPS = 1e-8
CALL = int(os.environ.get("BASS_GNN_CALL", "768"))          # gather positions per dma_gather call
G_OH = 16            # one-hot tiles built per DVE op
PHASE = os.environ.get("BASS_GNN_PHASE", "full")
TRACE = os.environ.get("BASS_GNN_TRACE", "0") == "1"

f32 = mybir.dt.float32
bf16 = mybir.dt.bfloat16
i16 = mybir.dt.int16
Alu = mybir.AluOpType
Act = mybir.ActivationFunctionType


# ----------------------------------------------------------------------------
# Host-side preprocessing: permutation, block assignment, edge streams.
# ----------------------------------------------------------------------------

def _prep(nodes, edges):
    src = edges[:, 0].astype(np.int64)
    dst = edges[:, 1].astype(np.int64)

    indeg = np.zeros(NP, np.int64)
    np.add.at(indeg, dst, 1)
    order = np.argsort(-indeg, kind="stable")
    # round-robin by in-degree over 784 global blocks -> balanced block loads
    gblock = np.empty(NP, np.int64)
    gslot = np.empty(NP, np.int64)
    gblock[order] = np.arange(NP) % NBG
    gslot[order] = np.arange(NP) // NBG

    # block -> core is FIXED (g % W) so each node's gather chunk (= its core
    # pair) is known exactly; then align per-(block,chunk) tile needs across
    # cores by sorting each core's blocks by their tile-need pattern.
    core_of_gblock = np.arange(NBG) % W
    chunk_n = core_of_gblock[gblock] // 2       # chunk of node as gather src
    E0 = np.zeros((NBG, NCHUNK), np.int64)
    np.add.at(E0, (gblock[dst], chunk_n[src]), 1)
    t0 = np.ceil(E0 / 128).astype(np.int64)
    key = (t0 * (5 ** np.arange(NCHUNK))[::-1]).sum(1) * 1000 + E0.sum(1) // 16
    lslot_of_gblock = np.empty(NBG, np.int64)
    for c in range(W):
        mine = np.where(core_of_gblock == c)[0]
        order_c = mine[np.argsort(key[mine], kind="stable")]
        lslot_of_gblock[order_c] = np.arange(len(order_c))

    # final permutation: pi(n) = core*SH + slot_within_block*NBLK + local_block
    # (p-major layout so SBUF slab [128, 98, 64] maps contiguously to DRAM rows)
    c_n = core_of_gblock[gblock]
    b_n = lslot_of_gblock[gblock]
    p_n = gslot
    pi = c_n * SH + p_n * NBLK + b_n

    # exact per-(core, local block, chunk) edge counts under final assignment
    e_dc = c_n[dst]
    e_db = b_n[dst]
    e_dp = p_n[dst]
    e_chunk = pi[src] // CH
    Ec = np.zeros((W, NBLK, NCHUNK), np.int64)
    np.add.at(Ec, (e_dc, e_db, e_chunk), 1)
    S_sub = np.ceil(Ec / 128).astype(np.int64).max(0)  # [NBLK, NCHUNK]
    S_sub = np.maximum(S_sub, 1)   # keep schedule uniform & non-degenerate
    assert S_sub.max() <= CALL // 128

    # edge order: (core, block, chunk, src)
    ek = np.lexsort((pi[src], e_chunk, e_db, e_dc))
    src_pi = pi[src][ek]
    dloc_e = e_dp[ek]
    key_cbk = (e_dc[ek] * NBLK + e_db[ek]) * NCHUNK + e_chunk[ek]
    counts = np.zeros(W * NBLK * NCHUNK + 1, np.int64)
    np.add.at(counts, key_cbk + 1, 1)
    starts = np.cumsum(counts)

    # gather-stream offsets: chunk-major, then block, then tile
    TPOS = int(S_sub.sum() * 128)
    NT = TPOS // 128
    sched_off = np.zeros((NCHUNK, NBLK), np.int64)
    off = 0
    for k in range(NCHUNK):
        for b in range(NBLK):
            sched_off[k, b] = off
            off += int(S_sub[b, k]) * 128
    assert off == TPOS

    gidx = np.zeros((W, TPOS), np.int32)
    dloc = np.full((W, TPOS), -1.0, np.float32)   # stream order
    for c in range(W):
        for k in range(NCHUNK):
            for b in range(NBLK):
                i0 = starts[(c * NBLK + b) * NCHUNK + k]
                i1 = starts[(c * NBLK + b) * NCHUNK + k + 1]
                n = i1 - i0
                o = sched_off[k, b]
                assert n <= S_sub[b, k] * 128
                gidx[c, o:o + n] = src_pi[i0:i1] - k * CH
                dloc[c, o:o + n] = dloc_e[i0:i1]
    assert gidx.min() >= 0 and gidx.max() < CH

    # wrapped-16 int16 idx layout (stream order)
    gidx16 = np.zeros((W, 128, TPOS // 16), np.int16)
    for c in range(W):
        g = gidx[c].reshape(-1, 16).T.astype(np.int16)
        gidx16[c] = np.tile(g, (8, 1))

    # consumption order: block-major (b, k, s) -> stream position of each tile
    cons_pos = []
    for b in range(NBLK):
        for k in range(NCHUNK):
            for s in range(int(S_sub[b, k])):
                cons_pos.append(int(sched_off[k, b]) + s * 128)
    cons_pos = np.array(cons_pos, np.int64)
    assert len(cons_pos) == NT
    # dloc in consumption-tile-column layout [128, NT]
    dloc_s = dloc.reshape(W, NT, 128)        # stream tile t at rows [t]
    stream_tile = cons_pos // 128
    dloc_c = dloc_s[:, stream_tile, :].transpose(0, 2, 1).astype(np.float32).copy()

    # per-chunk call plan, issued round-robin across chunks
    ncalls_k = []
    base_k = []
    for k in range(NCHUNK):
        npos = int(S_sub[:, k].sum() * 128)
        base_k.append(int(sched_off[k, 0]))
        ncalls_k.append((npos + CALL - 1) // CALL)
    calls = []   # (k, stream_off, npos) in issue order
    for ci in range(max(ncalls_k)):
        for k in range(NCHUNK):
            if ci < ncalls_k[k]:
                npos_k = int(S_sub[:, k].sum() * 128)
                o = base_k[k] + ci * CALL
                n = min(CALL, base_k[k] + npos_k - o)
                calls.append((k, o, n))

    return {
        "pi": pi, "S_sub": S_sub, "TPOS": TPOS, "NT": NT,
        "gidx16": gidx16, "dloc_c": dloc_c, "calls": calls,
        "sched_off": sched_off, "cons_pos": cons_pos,
    }


# ----------------------------------------------------------------------------
# Kernel builder
# ----------------------------------------------------------------------------

def _emit_hl(nc, tcx, emb_slab, shard_hl):
    """Split fp32 slab into bf16 hi|lo and write the interleaved shard rows."""
    with tcx.tile_pool(name="hl", bufs=1) as hp:
        hl = hp.tile([128, NBLK, 2 * D], bf16, tag="hl")
        tmp = hp.tile([128, NBLK, D], f32, tag="tmp")
        nc.vector.tensor_copy(out=hl[:, :, :D], in_=emb_slab[:])
        nc.vector.tensor_copy(out=tmp[:], in_=hl[:, :, :D])
        nc.vector.tensor_tensor(out=tmp[:], in0=emb_slab[:], in1=tmp[:],
                                op=Alu.subtract)
        nc.vector.tensor_copy(out=hl[:, :, D:], in_=tmp[:])
        nc.sync.dma_start(shard_hl[:, :], hl[:])


def _build(meta):
    S_sub = meta["S_sub"]
    TPOS = meta["TPOS"]
    NT = meta["NT"]
    calls = meta["calls"]
    cons_pos = meta["cons_pos"]

    nc = bacc.Bacc("TRN2", debug=False, num_swdge_queues=4)

    ne_tab = nc.dram_tensor("ne_tab", [128, D], f32, kind="ExternalInput")
    type_idx = nc.dram_tensor("type_idx", [128, SH // 16], i16, kind="ExternalInput")
    featT_in = nc.dram_tensor("featT", [DF + 1, SH], f32, kind="ExternalInput")
    wft_in = nc.dram_tensor("wft", [DF + 1, D], f32, kind="ExternalInput")
    wct_in = nc.dram_tensor("wct", [D, D], f32, kind="ExternalInput")
    cb_in = nc.dram_tensor("cb_col", [D, 1], f32, kind="ExternalInput")
    p_in = nc.dram_tensor("p_col", [D, 1], f32, kind="ExternalInput")
    ones_in = nc.dram_tensor("ones_col", [D, 1], f32, kind="ExternalInput")
    onesrow_in = nc.dram_tensor("ones_row", [1, 128], f32, kind="ExternalInput")
    ident_in = nc.dram_tensor("ident", [128, 128], f32, kind="ExternalInput")
    iota_in = nc.dram_tensor("iota_rep", [128, G_OH * 128], bf16, kind="ExternalInput")
    gidx_in = nc.dram_tensor("gidx16", [128, TPOS // 16], i16, kind="ExternalInput")
    dloc_in = nc.dram_tensor("dloc_c", [128, NT], f32, kind="ExternalInput")

    score_out = nc.dram_tensor("score", [128, NBLK], f32, kind="ExternalOutput")
    if PHASE in ("emb0", "agg", "emb1"):
        dbg_out = nc.dram_tensor("dbg", [SH, D], f32, kind="ExternalOutput")

    shard_hl = nc.dram_tensor("shard_hl", [SH, 2 * D], bf16)
    table_hl = nc.dram_tensor("table_hl", [NP, 2 * D], bf16, addr_space="Shared")

    import contextlib

    def allgather(tcx):
        with tcx.tile_critical():
            with contextlib.ExitStack() as st:
                cc = st.enter_context(nc.semaphore())
                nc.gpsimd.collective_compute(
                    "AllGather", Alu.bypass,
                    replica_groups=[list(range(W))],
                    ins=[shard_hl.ap().opt()],
                    outs=[table_hl.ap().opt()],
                ).then_inc(cc, 1)
                nc.gpsimd.wait_ge(cc, 1)

    # consumption-tile schedule per block: (first_tile_index, total_tiles)
    S_tot = S_sub.sum(1).astype(np.int64)
    blk_t0 = np.concatenate([[0], np.cumsum(S_tot)])[:-1]

    with tile.TileContext(nc) as tc:
        nc.gpsimd.load_library(library_config.mlp)
        with contextlib.ExitStack() as big:
            cpool = big.enter_context(tc.tile_pool(name="consts", bufs=1))
            slabs = big.enter_context(tc.tile_pool(name="slabs", bufs=1))

            wft_sb = cpool.tile([DF + 1, D], f32, tag="wft")
            nc.sync.dma_start(wft_sb[:], wft_in[:])
            wct_sb = cpool.tile([D, D], f32, tag="wct")
            nc.sync.dma_start(wct_sb[:], wct_in[:])
            cb_sb = cpool.tile([D, 1], f32, tag="cb")
            nc.sync.dma_start(cb_sb[:], cb_in[:])
            p_sb = cpool.tile([D, 1], f32, tag="p")
            nc.sync.dma_start(p_sb[:], p_in[:])
            ones_sb = cpool.tile([D, 1], f32, tag="ones")
            nc.sync.dma_start(ones_sb[:], ones_in[:])
            onesrow_sb = cpool.tile([1, 128], f32, tag="onesrow")
            nc.sync.dma_start(onesrow_sb[:], onesrow_in[:])
            ident_sb = cpool.tile([128, 128], f32, tag="ident")
            nc.sync.dma_start(ident_sb[:], ident_in[:])
            iota_sb = cpool.tile([128, G_OH * 128], bf16, tag="iota")
            nc.sync.dma_start(iota_sb[:], iota_in[:])
            dloc_sb = cpool.tile([128, NT], f32, tag="dloc")
            nc.sync.dma_start(dloc_sb[:], dloc_in[:])
            gidx_sb = cpool.tile([128, TPOS // 16], i16, tag="gidx")
            nc.sync.dma_start(gidx_sb[:], gidx_in[:])

            emb_slab = slabs.tile([128, NBLK, D], f32, tag="emb")
            agg_slab = slabs.tile([128, NBLK, D], f32, tag="agg")
            num_slab = slabs.tile([128, NBLK], f32, tag="num")
            nsq_slab = slabs.tile([128, NBLK], f32, tag="nsq")

            # ---------------- Phase 0: initial embeddings -------------------
            with tc.tile_pool(name="ph0", bufs=1) as ph0, \
                 tc.tile_pool(name="ph0ps", bufs=8, space="PSUM") as ph0ps:
                tix = ph0.tile([128, SH // 16], i16, tag="tix")
                nc.sync.dma_start(tix[:], type_idx[:])
                featT_sb = ph0.tile([DF + 1, SH], f32, tag="featT")
                nc.sync.dma_start(featT_sb[:], featT_in[:])
                tg_blks = [25, 25, 24, 24]
                tb = 0
                for q in range(4):
                    npos = tg_blks[q] * 128
                    nc.gpsimd.dma_gather(
                        emb_slab[:, tb:tb + tg_blks[q], :], ne_tab[:],
                        tix[:, tb * 8:(tb + tg_blks[q]) * 8],
                        npos, npos, D,
                        single_packet=False, queue_num=q)
                    tb += tg_blks[q]
                for b in range(NBLK):
                    ps = ph0ps.tile([128, D], f32, tag="p0")
                    nc.tensor.matmul(
                        ps[:], featT_sb[:, b * 128:(b + 1) * 128], wft_sb[:],
                        start=True, stop=True)
                    nc.vector.tensor_tensor(
                        out=emb_slab[:, b, :], in0=emb_slab[:, b, :], in1=ps[:],
                        op=Alu.add)
            _emit_hl(nc, tc, emb_slab, shard_hl)
            allgather(tc)

            if PHASE == "emb0":
                nc.sync.dma_start(dbg_out[:, :], emb_slab[:])

            # ---------------- rounds -----------------------------------------
            nrounds = 0 if PHASE == "emb0" else (1 if PHASE in ("agg", "emb1") else 2)
            for r in range(nrounds):
                last = r == nrounds - 1 and PHASE == "full"
                with tc.tile_pool(name=f"g{r}", bufs=int(os.environ.get("BASS_GNN_GBUFS", "32"))) as gpool, \
                     tc.tile_pool(name=f"oh{r}", bufs=6) as ohpool, \
                     tc.tile_pool(name=f"ps{r}", bufs=8, space="PSUM") as pspool:
                    call_tiles = {}
                    for (k, o, npos) in calls:
                        gt = gpool.tile([128, CALL // 128, 2 * D], bf16, tag="gbuf")
                        nc.gpsimd.dma_gather(
                            gt[:, :npos // 128, :],
                            table_hl[k * CH:(k + 1) * CH, :],
                            gidx_sb[:, o // 16:(o + npos) // 16],
                            npos, npos, 2 * D,
                            single_packet=False, queue_num=k)
                        call_tiles[o] = (gt, npos)
                    call_offs = sorted(call_tiles.keys())

                    import bisect

                    def tile_at(pos):
                        j = bisect.bisect_right(call_offs, pos) - 1
                        o = call_offs[j]
                        gt, npos = call_tiles[o]
                        assert o <= pos < o + npos
                        return gt[:, (pos - o) // 128, :]

                    oh_tiles = {}

                    def oh_at(t):
                        g0 = (t // G_OH) * G_OH
                        if g0 not in oh_tiles:
                            gsz = min(G_OH, NT - g0)
                            oh = ohpool.tile([128, G_OH, 128], bf16, tag="oh")
                            nc.vector.tensor_tensor(
                                out=oh[:, :gsz, :],
                                in0=iota_sb[:].rearrange(
                                    "p (g j) -> p g j", j=128)[:, :gsz, :],
                                in1=dloc_sb[:, g0:g0 + gsz].to_broadcast(
                                    [128, gsz, 128]),
                                op=Alu.is_equal)
                            oh_tiles[g0] = oh
                        return oh_tiles[g0][:, t - g0, :]

                    for b in range(NBLK):
                        stot = int(S_tot[b])
                        ps = pspool.tile([128, 2 * D], f32, tag="acc")
                        for j in range(stot):
                            t = int(blk_t0[b]) + j
                            nc.tensor.matmul(
                                ps[:], oh_at(t), tile_at(int(cons_pos[t])),
                                start=(j == 0), stop=(j == stot - 1))
                        nc.vector.reduce_sum(
                            out=agg_slab[:, b, :],
                            in_=ps[:].rearrange("p (h d) -> p d h", h=2),
                            axis=mybir.AxisListType.X)

                if PHASE == "agg" and r == 0:
                    nc.sync.dma_start(dbg_out[:, :], agg_slab[:])

                # x = emb + agg (in place into agg_slab)
                nc.vector.tensor_tensor(
                    out=agg_slab[:], in0=agg_slab[:], in1=emb_slab[:], op=Alu.add)

                # conv: relu(x @ W.T + b) via transposed layout; round 2 fuses
                # the cosine-score matmuls instead of transposing back.
                with tc.tile_pool(name=f"cv{r}", bufs=3) as cv, \
                     tc.tile_pool(name=f"cvps{r}", bufs=2, space="PSUM") as cvps:
                    for g0 in range(0, NBLK, 4):
                        blks = list(range(g0, min(g0 + 4, NBLK)))
                        wdt = 128 * len(blks)
                        pst = cvps.tile([D, 512], f32, tag="tr")
                        for j, b in enumerate(blks):
                            nc.tensor.transpose(
                                pst[:, j * 128:(j + 1) * 128],
                                agg_slab[:, b, :], ident_sb[:])
                        xT = cv.tile([D, 512], f32, tag="xT")
                        nc.vector.tensor_copy(out=xT[:, :wdt], in_=pst[:, :wdt])
                        zps = cvps.tile([D, 512], f32, tag="z")
                        nc.tensor.matmul(
                            zps[:, :wdt], wct_sb[:], xT[:, :wdt],
                            start=True, stop=True)
                        zr = cv.tile([D, 512], f32, tag="zr")
                        nc.vector.tensor_scalar(
                            zr[:, :wdt], zps[:, :wdt], cb_sb[:, :1], 0.0,
                            Alu.add, Alu.max)
                        if not last:
                            psb = cvps.tile([128, 4 * D], f32, tag="trb")
                            for j, b in enumerate(blks):
                                nc.tensor.transpose(
                                    psb[:, j * D:(j + 1) * D],
                                    zr[:, j * 128:(j + 1) * 128],
                                    ident_sb[:D, :D])
                            nc.vector.tensor_copy(
                                out=emb_slab[:, g0:g0 + len(blks), :],
                                in_=psb[:, :len(blks) * D])
                        else:
                            sq = cv.tile([D, 512], f32, tag="sq")
                            nc.vector.tensor_tensor(
                                out=sq[:, :wdt], in0=zr[:, :wdt],
                                in1=zr[:, :wdt], op=Alu.mult)
                            nps = cvps.tile([128, 4], f32, tag="nps")
                            qps = cvps.tile([128, 4], f32, tag="qps")
                            for j, b in enumerate(blks):
                                nc.tensor.matmul(
                                    nps[:, j:j + 1],
                                    zr[:, j * 128:(j + 1) * 128], p_sb[:],
                                    start=True, stop=True)
                                nc.tensor.matmul(
                                    qps[:, j:j + 1],
                                    sq[:, j * 128:(j + 1) * 128], ones_sb[:],
                                    start=True, stop=True)
                            nc.vector.tensor_copy(
                                out=num_slab[:, g0:g0 + len(blks)],
                                in_=nps[:, :len(blks)])
                            nc.vector.tensor_copy(
                                out=nsq_slab[:, g0:g0 + len(blks)],
                                in_=qps[:, :len(blks)])
                if not last:
                    _emit_hl(nc, tc, emb_slab, shard_hl)
                    if r == 0 and nrounds == 2:
                        allgather(tc)
                    if PHASE == "emb1" and r == 0:
                        nc.sync.dma_start(dbg_out[:, :], emb_slab[:])

            # ---------------- cosine scores ----------------------------------
            if PHASE == "full":
                with tc.tile_pool(name="cos", bufs=1) as cos, \
                     tc.tile_pool(name="cosps", bufs=2, space="PSUM") as cosps:
                    # pnorm = max(||p||, eps) replicated to [128,1]
                    psq = cos.tile([D, 1], f32, tag="psq")
                    nc.vector.tensor_tensor(
                        out=psq[:], in0=p_sb[:], in1=p_sb[:], op=Alu.mult)
                    pn_ps = cosps.tile([1, 1], f32, tag="pn")
                    nc.tensor.matmul(pn_ps[:], psq[:], ones_sb[:],
                                     start=True, stop=True)
                    pn_sb = cos.tile([1, 1], f32, tag="pnsb")
                    nc.scalar.activation(pn_sb[:], pn_ps[:], Act.Sqrt)
                    nc.vector.tensor_scalar(
                        pn_sb[:], pn_sb[:], EPS, None, Alu.max)
                    pnr_ps = cosps.tile([128, 1], f32, tag="pnr")
                    nc.tensor.matmul(pnr_ps[:], onesrow_sb[:], pn_sb[:],
                                     start=True, stop=True)
                    pnrep = cos.tile([128, 1], f32, tag="pnrep")
                    nc.vector.tensor_copy(out=pnrep[:], in_=pnr_ps[:])

                    norm = cos.tile([128, NBLK], f32, tag="norm")
                    nc.scalar.activation(norm[:], nsq_slab[:], Act.Sqrt)
                    nc.vector.tensor_scalar(
                        norm[:], norm[:], EPS, None, Alu.max)
                    nc.vector.tensor_scalar(
                        norm[:], norm[:], pnrep[:, :1], None, Alu.mult)
                    nc.vector.reciprocal(norm[:], norm[:])
                    nc.vector.tensor_tensor(
                        out=num_slab[:], in0=num_slab[:], in1=norm[:],
                        op=Alu.mult)
                    nc.sync.dma_start(score_out[:, :], num_slab[:])
            else:
                nc.vector.memset(num_slab[:, :1], 0.0)
                nc.sync.dma_start(score_out[:, :1], num_slab[:, :1])

    nc.compile()
    return nc


# ----------------------------------------------------------------------------
# Public entry
# ----------------------------------------------------------------------------

_cache = {}


def kernel(nodes, edges, features, node_emb, feat_W, feat_b,
           conv1_W, conv1_b, pattern_emb, pattern_id):
    import ml_dtypes

    nodes = np.asarray(nodes)
    edges = np.asarray(edges)
    features = np.asarray(features, np.float32)
    node_emb = np.asarray(node_emb, np.float32)
    feat_W = np.asarray(feat_W, np.float32)
    feat_b = np.asarray(feat_b, np.float32)
    conv1_W = np.asarray(conv1_W, np.float32)
    conv1_b = np.asarray(conv1_b, np.float32)
    pattern_emb = np.asarray(pattern_emb, np.float32)
    pid = int(np.asarray(pattern_id))

    meta = _prep(nodes, edges)
    pi = meta["pi"]

    key = (meta["TPOS"], meta["S_sub"].tobytes(), PHASE)
    if key not in _cache:
        _cache.clear()
        _cache[key] = _build(meta)
    nc = _cache[key]

    types_p = np.zeros(NP, np.int64)
    types_p[pi[:N]] = nodes.astype(np.int64)
    feat_p = np.zeros((NP, DF), np.float32)
    feat_p[pi[:N]] = features

    ne_tab = np.zeros((128, D), np.float32)
    ne_tab[:VOCAB] = node_emb
    wft = np.concatenate([feat_W.T, feat_b[None, :]], 0).astype(np.float32)
    wct = conv1_W.T.copy()
    cb_col = conv1_b.reshape(D, 1).copy()
    p_col = pattern_emb[pid].reshape(D, 1).copy()
    ones_col = np.ones((D, 1), np.float32)
    ones_row = np.ones((1, 128), np.float32)
    ident = np.eye(128, dtype=np.float32)
    iota_rep = np.broadcast_to(np.arange(128).astype(ml_dtypes.bfloat16),
                               (128, G_OH, 128)).reshape(128, G_OH * 128).copy()

    in_maps = []
    for c in range(W):
        rows = slice(c * SH, (c + 1) * SH)
        # table rows are p-major (row = p*NBLK + b) but dma_gather writes
        # stream position i to slab [i%128, i//128] = [p, b] -> feed the type
        # stream in (b, p) order so row p*NBLK+b lands at [p, b].
        tv = types_p[rows].reshape(128, NBLK).T.ravel()
        t16 = tv.reshape(-1, 16).T.astype(np.int16)
        t16 = np.tile(t16, (8, 1))
        fv = feat_p[rows].reshape(128, NBLK, DF).transpose(1, 0, 2)
        featT_c = fv.reshape(SH, DF).T
        featT_c = np.concatenate([featT_c, np.ones((1, SH), np.float32)], 0)
        in_maps.append({
            "ne_tab": ne_tab,
            "type_idx": t16,
            "featT": np.ascontiguousarray(featT_c),
            "wft": wft, "wct": wct,
            "cb_col": cb_col, "p_col": p_col, "ones_col": ones_col,
            "ones_row": ones_row, "ident": ident, "iota_rep": iota_rep,
            "gidx16": meta["gidx16"][c],
            "dloc_c": meta["dloc_c"][c],
        })

    tdir = os.environ.get("BASS_GNN_TRACE_DIR") or None
    res = run_bass_kernel_spmd(nc, in_maps, core_ids=list(range(W)),
                               trace=TRACE, tmpdir=tdir)
    kernel.last_results = res

    if PHASE != "full":
        dump = np.concatenate([res.results[c]["dbg"] for c in range(W)], 0)
        return dump

    out_p = np.empty(NP, np.float32)
    for c in range(W):
        s = res.results[c]["score"]
        out_p[c * SH:(c + 1) * SH] = s.reshape(SH)
    return out_p[pi[:N]]


# revision 27
# speedup vs baseline: 1.1026x; 1.0230x over previous
"""GNN message-passing kernel for Trainium2 (8 NeuronCores, Bass/Tile).

Implements:
  embeds = node_emb[nodes] + features @ feat_W.T + feat_b
  2 x { agg[dst] += embeds[src];  embeds = relu((embeds+agg) @ conv1_W.T + conv1_b) }
  out = cosine(embeds, pattern_emb[pattern_id])

Distribution: nodes are permuted (in-degree balanced) and sharded over 8 cores
(12544 nodes each).  Edges are partitioned by dst owner; messages are fetched
with GPSIMD dma_gather (4 SWDGE queues, one per table chunk) from a replicated
bf16 hi|lo DRAM table rebuilt each round with an ncfw AllGather.  Per 128-dst
block the segment-sum is a one-hot matmul accumulated in PSUM.
"""

import os
import sys

sys.path.insert(0, "/opt/trn_rl_repo")

import numpy as np

import concourse.bass as bass
import concourse.bacc as bacc
import concourse.mybir as mybir
import concourse.tile as tile
from concourse import library_config
from concourse.bass_utils import run_bass_kernel_spmd

N = 100000
NP = 100352          # padded: 8 * 12544
W = 8                # cores
SH = NP // W         # 12544 nodes per core
NBLK = SH // 128     # 98 blocks per core
NBG = NP // 128      # 784 global blocks
NCHUNK = 4
CH = NP // NCHUNK    # 25088 rows per gather chunk (int16-safe)
D = 64
DF = 10
VOCAB = 100
EPS = 1e-8
CALL = int(os.environ.get("BASS_GNN_CALL", "768"))          # gather positions per dma_gather call
G_OH = 16            # one-hot tiles built per DVE op
PHASE = os.environ.get("BASS_GNN_PHASE", "full")
TRACE = os.environ.get("BASS_GNN_TRACE", "0") == "1"

f32 = mybir.dt.float32
bf16 = mybir.dt.bfloat16
i16 = mybir.dt.int16
Alu = mybir.AluOpType
Act = mybir.ActivationFunctionType


# ----------------------------------------------------------------------------
# Host-side preprocessing: permutation, block assignment, edge streams.
# ----------------------------------------------------------------------------

def _prep(nodes, edges):
    src = edges[:, 0].astype(np.int64)
    dst = edges[:, 1].astype(np.int64)

    indeg = np.zeros(NP, np.int64)
    np.add.at(indeg, dst, 1)
    order = np.argsort(-indeg, kind="stable")
    # round-robin by in-degree over 784 global blocks -> balanced block loads
    gblock = np.empty(NP, np.int64)
    gslot = np.empty(NP, np.int64)
    gblock[order] = np.arange(NP) % NBG
    gslot[order] = np.arange(NP) // NBG

    # block -> core is FIXED (g % W) so each node's gather chunk (= its core
    # pair) is known exactly; then align per-(block,chunk) tile needs across
    # cores by sorting each core's blocks by their tile-need pattern.
    core_of_gblock = np.arange(NBG) % W
    chunk_n = core_of_gblock[gblock] // 2       # chunk of node as gather src
    E0 = np.zeros((NBG, NCHUNK), np.int64)
    np.add.at(E0, (gblock[dst], chunk_n[src]), 1)
    t0 = np.ceil(E0 / 128).astype(np.int64)
    key = (t0 * (5 ** np.arange(NCHUNK))[::-1]).sum(1) * 1000 + E0.sum(1) // 16
    lslot_of_gblock = np.empty(NBG, np.int64)
    for c in range(W):
        mine = np.where(core_of_gblock == c)[0]
        order_c = mine[np.argsort(key[mine], kind="stable")]
        lslot_of_gblock[order_c] = np.arange(len(order_c))

    # final permutation: pi(n) = core*SH + slot_within_block*NBLK + local_block
    # (p-major layout so SBUF slab [128, 98, 64] maps contiguously to DRAM rows)
    c_n = core_of_gblock[gblock]
    b_n = lslot_of_gblock[gblock]
    p_n = gslot
    pi = c_n * SH + p_n * NBLK + b_n

    # exact per-(core, local block, chunk) edge counts under final assignment
    e_dc = c_n[dst]
    e_db = b_n[dst]
    e_dp = p_n[dst]
    e_chunk = pi[src] // CH
    Ec = np.zeros((W, NBLK, NCHUNK), np.int64)
    np.add.at(Ec, (e_dc, e_db, e_chunk), 1)
    S_sub = np.ceil(Ec / 128).astype(np.int64).max(0)  # [NBLK, NCHUNK]
    S_sub = np.maximum(S_sub, 1)   # keep schedule uniform & non-degenerate
    assert S_sub.max() <= CALL // 128

    # edge order: (core, block, chunk, src)
    ek = np.lexsort((pi[src], e_chunk, e_db, e_dc))
    src_pi = pi[src][ek]
    dloc_e = e_dp[ek]
    key_cbk = (e_dc[ek] * NBLK + e_db[ek]) * NCHUNK + e_chunk[ek]
    counts = np.zeros(W * NBLK * NCHUNK + 1, np.int64)
    np.add.at(counts, key_cbk + 1, 1)
    starts = np.cumsum(counts)

    # gather-stream offsets: chunk-major, then block, then tile
    TPOS = int(S_sub.sum() * 128)
    NT = TPOS // 128
    sched_off = np.zeros((NCHUNK, NBLK), np.int64)
    off = 0
    for k in range(NCHUNK):
        for b in range(NBLK):
            sched_off[k, b] = off
            off += int(S_sub[b, k]) * 128
    assert off == TPOS

    gidx = np.zeros((W, TPOS), np.int32)
    dloc = np.full((W, TPOS), -1.0, np.float32)   # stream order
    for c in range(W):
        for k in range(NCHUNK):
            for b in range(NBLK):
                i0 = starts[(c * NBLK + b) * NCHUNK + k]
                i1 = starts[(c * NBLK + b) * NCHUNK + k + 1]
                n = i1 - i0
                o = sched_off[k, b]
                assert n <= S_sub[b, k] * 128
                gidx[c, o:o + n] = src_pi[i0:i1] - k * CH
                dloc[c, o:o + n] = dloc_e[i0:i1]
    assert gidx.min() >= 0 and gidx.max() < CH

    # wrapped-16 int16 idx layout (stream order)
    gidx16 = np.zeros((W, 128, TPOS // 16), np.int16)
    for c in range(W):
        g = gidx[c].reshape(-1, 16).T.astype(np.int16)
        gidx16[c] = np.tile(g, (8, 1))

    # consumption order: block-major (b, k, s) -> stream position of each tile
    cons_pos = []
    for b in range(NBLK):
        for k in range(NCHUNK):
            for s in range(int(S_sub[b, k])):
                cons_pos.append(int(sched_off[k, b]) + s * 128)
    cons_pos = np.array(cons_pos, np.int64)
    assert len(cons_pos) == NT
    # dloc in consumption-tile-column layout [128, NT]
    dloc_s = dloc.reshape(W, NT, 128)        # stream tile t at rows [t]
    stream_tile = cons_pos // 128
    dloc_c = dloc_s[:, stream_tile, :].transpose(0, 2, 1).astype(np.float32).copy()

    # per-chunk call plan, issued round-robin across chunks
    ncalls_k = []
    base_k = []
    for k in range(NCHUNK):
        npos = int(S_sub[:, k].sum() * 128)
        base_k.append(int(sched_off[k, 0]))
        ncalls_k.append((npos + CALL - 1) // CALL)
    calls = []   # (k, stream_off, npos) in issue order
    for ci in range(max(ncalls_k)):
        for k in range(NCHUNK):
            if ci < ncalls_k[k]:
                npos_k = int(S_sub[:, k].sum() * 128)
                o = base_k[k] + ci * CALL
                n = min(CALL, base_k[k] + npos_k - o)
                calls.append((k, o, n))

    return {
        "pi": pi, "S_sub": S_sub, "TPOS": TPOS, "NT": NT,
        "gidx16": gidx16, "dloc_c": dloc_c, "calls": calls,
        "sched_off": sched_off, "cons_pos": cons_pos,
    }


# ----------------------------------------------------------------------------
# Kernel builder
# ----------------------------------------------------------------------------

def _emit_hl(nc, tcx, emb_slab, shard_hl):
    """Split fp32 slab into bf16 hi|lo and write the interleaved shard rows."""
    with tcx.tile_pool(name="hl", bufs=1) as hp:
        hl = hp.tile([128, NBLK, 2 * D], bf16, tag="hl")
        tmp = hp.tile([128, NBLK, D], f32, tag="tmp")
        nc.vector.tensor_copy(out=hl[:, :, :D], in_=emb_slab[:])
        nc.vector.tensor_copy(out=tmp[:], in_=hl[:, :, :D])
        nc.vector.tensor_tensor(out=tmp[:], in0=emb_slab[:], in1=tmp[:],
                                op=Alu.subtract)
        nc.vector.tensor_copy(out=hl[:, :, D:], in_=tmp[:])
        nc.sync.dma_start(shard_hl[:, :], hl[:])


def _build(meta):
    S_sub = meta["S_sub"]
    TPOS = meta["TPOS"]
    NT = meta["NT"]
    calls = meta["calls"]
    cons_pos = meta["cons_pos"]

    nc = bacc.Bacc("TRN2", debug=False, num_swdge_queues=4)

    ne_tab = nc.dram_tensor("ne_tab", [128, D], f32, kind="ExternalInput")
    type_idx = nc.dram_tensor("type_idx", [128, SH // 16], i16, kind="ExternalInput")
    featT_in = nc.dram_tensor("featT", [DF + 1, SH], f32, kind="ExternalInput")
    wft_in = nc.dram_tensor("wft", [DF + 1, D], f32, kind="ExternalInput")
    wct_in = nc.dram_tensor("wct", [D, D], f32, kind="ExternalInput")
    cb_in = nc.dram_tensor("cb_col", [D, 1], f32, kind="ExternalInput")
    p_in = nc.dram_tensor("p_col", [D, 1], f32, kind="ExternalInput")
    ones_in = nc.dram_tensor("ones_col", [D, 1], f32, kind="ExternalInput")
    onesrow_in = nc.dram_tensor("ones_row", [1, 128], f32, kind="ExternalInput")
    ident_in = nc.dram_tensor("ident", [128, 128], f32, kind="ExternalInput")
    iota_in = nc.dram_tensor("iota_rep", [128, G_OH * 128], bf16, kind="ExternalInput")
    gidx_in = nc.dram_tensor("gidx16", [128, TPOS // 16], i16, kind="ExternalInput")
    dloc_in = nc.dram_tensor("dloc_c", [128, NT], f32, kind="ExternalInput")

    score_out = nc.dram_tensor("score", [128, NBLK], f32, kind="ExternalOutput")
    if PHASE in ("emb0", "agg", "emb1"):
        dbg_out = nc.dram_tensor("dbg", [SH, D], f32, kind="ExternalOutput")

    shard_hl = nc.dram_tensor("shard_hl", [SH, 2 * D], bf16)
    table_hl = nc.dram_tensor("table_hl", [NP, 2 * D], bf16, addr_space="Shared")

    import contextlib

    def allgather(tcx):
        with tcx.tile_critical():
            with contextlib.ExitStack() as st:
                cc = st.enter_context(nc.semaphore())
                nc.gpsimd.collective_compute(
                    "AllGather", Alu.bypass,
                    replica_groups=[list(range(W))],
                    ins=[shard_hl.ap().opt()],
                    outs=[table_hl.ap().opt()],
                ).then_inc(cc, 1)
                nc.gpsimd.wait_ge(cc, 1)

    # consumption-tile schedule per block: (first_tile_index, total_tiles)
    S_tot = S_sub.sum(1).astype(np.int64)
    blk_t0 = np.concatenate([[0], np.cumsum(S_tot)])[:-1]

    with tile.TileContext(nc) as tc:
        nc.gpsimd.load_library(library_config.mlp)
        with contextlib.ExitStack() as big:
            cpool = big.enter_context(tc.tile_pool(name="consts", bufs=1))
            slabs = big.enter_context(tc.tile_pool(name="slabs", bufs=1))

            wft_sb = cpool.tile([DF + 1, D], f32, tag="wft")
            nc.sync.dma_start(wft_sb[:], wft_in[:])
            wct_sb = cpool.tile([D, D], f32, tag="wct")
            nc.sync.dma_start(wct_sb[:], wct_in[:])
            cb_sb = cpool.tile([D, 1], f32, tag="cb")
            nc.sync.dma_start(cb_sb[:], cb_in[:])
            p_sb = cpool.tile([D, 1], f32, tag="p")
            nc.sync.dma_start(p_sb[:], p_in[:])
            ones_sb = cpool.tile([D, 1], f32, tag="ones")
            nc.sync.dma_start(ones_sb[:], ones_in[:])
            onesrow_sb = cpool.tile([1, 128], f32, tag="onesrow")
            nc.sync.dma_start(onesrow_sb[:], onesrow_in[:])
            ident_sb = cpool.tile([128, 128], f32, tag="ident")
            nc.sync.dma_start(ident_sb[:], ident_in[:])
            iota_sb = cpool.tile([128, G_OH * 128], bf16, tag="iota")
            nc.sync.dma_start(iota_sb[:], iota_in[:])
            dloc_sb = cpool.tile([128, NT], f32, tag="dloc")
            nc.sync.dma_start(dloc_sb[:], dloc_in[:])
            gidx_sb = cpool.tile([128, TPOS // 16], i16, tag="gidx")
            nc.sync.dma_start(gidx_sb[:], gidx_in[:])

            emb_slab = slabs.tile([128, NBLK, D], f32, tag="emb")
            agg_slab = slabs.tile([128, NBLK, D], f32, tag="agg")
            num_slab = slabs.tile([128, NBLK], f32, tag="num")
            nsq_slab = slabs.tile([128, NBLK], f32, tag="nsq")

            # ---------------- Phase 0: initial embeddings -------------------
            with tc.tile_pool(name="ph0", bufs=1) as ph0, \
                 tc.tile_pool(name="ph0ps", bufs=8, space="PSUM") as ph0ps:
                tix = ph0.tile([128, SH // 16], i16, tag="tix")
                nc.sync.dma_start(tix[:], type_idx[:])
                featT_sb = ph0.tile([DF + 1, SH], f32, tag="featT")
                nc.sync.dma_start(featT_sb[:], featT_in[:])
                tg_blks = [25, 25, 24, 24]
                tb = 0
                for q in range(4):
                    npos = tg_blks[q] * 128
                    nc.gpsimd.dma_gather(
                        emb_slab[:, tb:tb + tg_blks[q], :], ne_tab[:],
                        tix[:, tb * 8:(tb + tg_blks[q]) * 8],
                        npos, npos, D,
                        single_packet=False, queue_num=q)
                    tb += tg_blks[q]
                for b in range(NBLK):
                    ps = ph0ps.tile([128, D], f32, tag="p0")
                    nc.tensor.matmul(
                        ps[:], featT_sb[:, b * 128:(b + 1) * 128], wft_sb[:],
                        start=True, stop=True)
                    nc.vector.tensor_tensor(
                        out=emb_slab[:, b, :], in0=emb_slab[:, b, :], in1=ps[:],
                        op=Alu.add)
            _emit_hl(nc, tc, emb_slab, shard_hl)
            allgather(tc)

            if PHASE == "emb0":
                nc.sync.dma_start(dbg_out[:, :], emb_slab[:])

            # ---------------- rounds -----------------------------------------
            nrounds = 0 if PHASE == "emb0" else (1 if PHASE in ("agg", "emb1") else 2)
            for r in range(nrounds):
                last = r == nrounds - 1 and PHASE == "full"
                with tc.tile_pool(name=f"g{r}", bufs=int(os.environ.get("BASS_GNN_GBUFS", "32"))) as gpool, \
                     tc.tile_pool(name=f"oh{r}", bufs=4) as ohpool, \
                     tc.tile_pool(name=f"ps{r}", bufs=8, space="PSUM") as pspool:
                    call_tiles = {}
                    for (k, o, npos) in calls:
                        gt = gpool.tile([128, CALL // 128, 2 * D], bf16, tag="gbuf")
                        nc.gpsimd.dma_gather(
                            gt[:, :npos // 128, :],
                            table_hl[k * CH:(k + 1) * CH, :],
                            gidx_sb[:, o // 16:(o + npos) // 16],
                            npos, npos, 2 * D,
                            single_packet=False, queue_num=k)
                        call_tiles[o] = (gt, npos)
                    call_offs = sorted(call_tiles.keys())

                    import bisect

                    def tile_at(pos):
                        j = bisect.bisect_right(call_offs, pos) - 1
                        o = call_offs[j]
                        gt, npos = call_tiles[o]
                        assert o <= pos < o + npos
                        return gt[:, (pos - o) // 128, :]

                    oh_tiles = {}

                    def oh_at(t):
                        g0 = (t // G_OH) * G_OH
                        if g0 not in oh_tiles:
                            gsz = min(G_OH, NT - g0)
                            oh = ohpool.tile([128, G_OH, 128], bf16, tag="oh")
                            nc.vector.tensor_tensor(
                                out=oh[:, :gsz, :],
                                in0=iota_sb[:].rearrange(
                                    "p (g j) -> p g j", j=128)[:, :gsz, :],
                                in1=dloc_sb[:, g0:g0 + gsz].to_broadcast(
                                    [128, gsz, 128]),
                                op=Alu.is_equal)
                            oh_tiles[g0] = oh
                        return oh_tiles[g0][:, t - g0, :]

                    for b in range(NBLK):
                        stot = int(S_tot[b])
                        ps = pspool.tile([128, 2 * D], f32, tag="acc")
                        for j in range(stot):
                            t = int(blk_t0[b]) + j
                            nc.tensor.matmul(
                                ps[:], oh_at(t), tile_at(int(cons_pos[t])),
                                start=(j == 0), stop=(j == stot - 1))
                        nc.vector.reduce_sum(
                            out=agg_slab[:, b, :],
                            in_=ps[:].rearrange("p (h d) -> p d h", h=2),
                            axis=mybir.AxisListType.X)

                if PHASE == "agg" and r == 0:
                    nc.sync.dma_start(dbg_out[:, :], agg_slab[:])

                # x = emb + agg (in place into agg_slab)
                nc.vector.tensor_tensor(
                    out=agg_slab[:], in0=agg_slab[:], in1=emb_slab[:], op=Alu.add)

                # conv: relu(x @ W.T + b) via transposed layout; round 2 fuses
                # the cosine-score matmuls instead of transposing back.
                with tc.tile_pool(name=f"cv{r}", bufs=3) as cv, \
                     tc.tile_pool(name=f"cvps{r}", bufs=2, space="PSUM") as cvps:
                    for g0 in range(0, NBLK, 4):
                        blks = list(range(g0, min(g0 + 4, NBLK)))
                        wdt = 128 * len(blks)
                        pst = cvps.tile([D, 512], f32, tag="tr")
                        for j, b in enumerate(blks):
                            nc.tensor.transpose(
                                pst[:, j * 128:(j + 1) * 128],
                                agg_slab[:, b, :], ident_sb[:])
                        xT = cv.tile([D, 512], f32, tag="xT")
                        nc.vector.tensor_copy(out=xT[:, :wdt], in_=pst[:, :wdt])
                        zps = cvps.tile([D, 512], f32, tag="z")
                        nc.tensor.matmul(
                            zps[:, :wdt], wct_sb[:], xT[:, :wdt],
                            start=True, stop=True)
                        zr = cv.tile([D, 512], f32, tag="zr")
                        nc.vector.tensor_scalar(
                            zr[:, :wdt], zps[:, :wdt], cb_sb[:, :1], 0.0,
                            Alu.add, Alu.max)
                        if not last:
                            psb = cvps.tile([128, 4 * D], f32, tag="trb")
                            for j, b in enumerate(blks):
                                nc.tensor.transpose(
                                    psb[:, j * D:(j + 1) * D],
                                    zr[:, j * 128:(j + 1) * 128],
                                    ident_sb[:D, :D])
                            nc.vector.tensor_copy(
                                out=emb_slab[:, g0:g0 + len(blks), :],
                                in_=psb[:, :len(blks) * D])
                        else:
                            sq = cv.tile([D, 512], f32, tag="sq")
                            nc.vector.tensor_tensor(
                                out=sq[:, :wdt], in0=zr[:, :wdt],
                                in1=zr[:, :wdt], op=Alu.mult)
                            nps = cvps.tile([128, 4], f32, tag="nps")
                            qps = cvps.tile([128, 4], f32, tag="qps")
                            for j, b in enumerate(blks):
                                nc.tensor.matmul(
                                    nps[:, j:j + 1],
                                    zr[:, j * 128:(j + 1) * 128], p_sb[:],
                                    start=True, stop=True)
                                nc.tensor.matmul(
                                    qps[:, j:j + 1],
                                    sq[:, j * 128:(j + 1) * 128], ones_sb[:],
                                    start=True, stop=True)
                            nc.vector.tensor_copy(
                                out=num_slab[:, g0:g0 + len(blks)],
                                in_=nps[:, :len(blks)])
                            nc.vector.tensor_copy(
                                out=nsq_slab[:, g0:g0 + len(blks)],
                                in_=qps[:, :len(blks)])
                if not last:
                    _emit_hl(nc, tc, emb_slab, shard_hl)
                    if r == 0 and nrounds == 2:
                        allgather(tc)
                    if PHASE == "emb1" and r == 0:
                        nc.sync.dma_start(dbg_out[:, :], emb_slab[:])

            # ---------------- cosine scores ----------------------------------
            if PHASE == "full":
                with tc.tile_pool(name="cos", bufs=1) as cos, \
                     tc.tile_pool(name="cosps", bufs=2, space="PSUM") as cosps:
                    # pnorm = max(||p||, eps) replicated to [128,1]
                    psq = cos.tile([D, 1], f32, tag="psq")
                    nc.vector.tensor_tensor(
                        out=psq[:], in0=p_sb[:], in1=p_sb[:], op=Alu.mult)
                    pn_ps = cosps.tile([1, 1], f32, tag="pn")
                    nc.tensor.matmul(pn_ps[:], psq[:], ones_sb[:],
                                     start=True, stop=True)
                    pn_sb = cos.tile([1, 1], f32, tag="pnsb")
                    nc.scalar.activation(pn_sb[:], pn_ps[:], Act.Sqrt)
                    nc.vector.tensor_scalar(
                        pn_sb[:], pn_sb[:], EPS, None, Alu.max)
                    pnr_ps = cosps.tile([128, 1], f32, tag="pnr")
                    nc.tensor.matmul(pnr_ps[:], onesrow_sb[:], pn_sb[:],
                                     start=True, stop=True)
                    pnrep = cos.tile([128, 1], f32, tag="pnrep")
                    nc.vector.tensor_copy(out=pnrep[:], in_=pnr_ps[:])

                    norm = cos.tile([128, NBLK], f32, tag="norm")
                    nc.scalar.activation(norm[:], nsq_slab[:], Act.Sqrt)
                    nc.vector.tensor_scalar(
                        norm[:], norm[:], EPS, None, Alu.max)
                    nc.vector.tensor_scalar(
                        norm[:], norm[:], pnrep[:, :1], None, Alu.mult)
                    nc.vector.reciprocal(norm[:], norm[:])
                    nc.vector.tensor_tensor(
                        out=num_slab[:], in0=num_slab[:], in1=norm[:],
                        op=Alu.mult)
                    nc.sync.dma_start(score_out[:, :], num_slab[:])
            else:
                nc.vector.memset(num_slab[:, :1], 0.0)
                nc.sync.dma_start(score_out[:, :1], num_slab[:, :1])

    nc.compile()
    return nc


# ----------------------------------------------------------------------------
# Public entry
# ----------------------------------------------------------------------------

_cache = {}


def kernel(nodes, edges, features, node_emb, feat_W, feat_b,
           conv1_W, conv1_b, pattern_emb, pattern_id):
    import ml_dtypes

    nodes = np.asarray(nodes)
    edges = np.asarray(edges)
    features = np.asarray(features, np.float32)
    node_emb = np.asarray(node_emb, np.float32)
    feat_W = np.asarray(feat_W, np.float32)
    feat_b = np.asarray(feat_b, np.float32)
    conv1_W = np.asarray(conv1_W, np.float32)
    conv1_b = np.asarray(conv1_b, np.float32)
    pattern_emb = np.asarray(pattern_emb, np.float32)
    pid = int(np.asarray(pattern_id))

    meta = _prep(nodes, edges)
    pi = meta["pi"]

    key = (meta["TPOS"], meta["S_sub"].tobytes(), PHASE)
    if key not in _cache:
        _cache.clear()
        _cache[key] = _build(meta)
    nc = _cache[key]

    types_p = np.zeros(NP, np.int64)
    types_p[pi[:N]] = nodes.astype(np.int64)
    feat_p = np.zeros((NP, DF), np.float32)
    feat_p[pi[:N]] = features

    ne_tab = np.zeros((128, D), np.float32)
    ne_tab[:VOCAB] = node_emb
    wft = np.concatenate([feat_W.T, feat_b[None, :]], 0).astype(np.float32)
    wct = conv1_W.T.copy()
    cb_col = conv1_b.reshape(D, 1).copy()
    p_col = pattern_emb[pid].reshape(D, 1).copy()
    ones_col = np.ones((D, 1), np.float32)
    ones_row = np.ones((1, 128), np.float32)
    ident = np.eye(128, dtype=np.float32)
    iota_rep = np.broadcast_to(np.arange(128).astype(ml_dtypes.bfloat16),
                               (128, G_OH, 128)).reshape(128, G_OH * 128).copy()

    in_maps = []
    for c in range(W):
        rows = slice(c * SH, (c + 1) * SH)
        # table rows are p-major (row = p*NBLK + b) but dma_gather writes
        # stream position i to slab [i%128, i//128] = [p, b] -> feed the type
        # stream in (b, p) order so row p*NBLK+b lands at [p, b].
        tv = types_p[rows].reshape(128, NBLK).T.ravel()
        t16 = tv.reshape(-1, 16).T.astype(np.int16)
        t16 = np.tile(t16, (8, 1))
        fv = feat_p[rows].reshape(128, NBLK, DF).transpose(1, 0, 2)
        featT_c = fv.reshape(SH, DF).T
        featT_c = np.concatenate([featT_c, np.ones((1, SH), np.float32)], 0)
        in_maps.append({
            "ne_tab": ne_tab,
            "type_idx": t16,
            "featT": np.ascontiguousarray(featT_c),
            "wft": wft, "wct": wct,
            "cb_col": cb_col, "p_col": p_col, "ones_col": ones_col,
            "ones_row": ones_row, "ident": ident, "iota_rep": iota_rep,
            "gidx16": meta["gidx16"][c],
            "dloc_c": meta["dloc_c"][c],
        })

    tdir = os.environ.get("BASS_GNN_TRACE_DIR") or None
    res = run_bass_kernel_spmd(nc, in_maps, core_ids=list(range(W)),
                               trace=TRACE, tmpdir=tdir)
    kernel.last_results = res

    if PHASE != "full":
        dump = np.concatenate([res.results[c]["dbg"] for c in range(W)], 0)
        return dump

    out_p = np.empty(NP, np.float32)
    for c in range(W):
        s = res.results[c]["score"]
        out_p[c * SH:(c + 1) * SH] = s.reshape(SH)
    return out_p[pi[:N]]
